# revision 1
# baseline (speedup 1.0000x reference)
"""Trainium2 Bass kernel for a GPT-2 style transformer block (pre-LN, no mask).

Reference shapes: x [B=2, T=2048, C=1024], H=16 heads, MLP hidden 4C=4096.

Sharding (8 NeuronCores): data-parallel over B (cores 0-3 -> batch 0,
cores 4-7 -> batch 1); within each 4-core group the 2048 query rows are
split 512 per core. Every core redundantly computes K and V for its full
batch from a replicated (rotated) copy of x, so no collectives are needed.

v2: fp8 (e4m3) DoubleRow matmuls for QKV / attn-proj / MLP (2 fp8 MACs
per PE cell per cycle, 256-deep contraction), fp8 softmax probabilities
feeding a DoubleRow P@V, and the exp stream split between the scalar
engine (hardware Exp) and the vector engine (Schraudolph int-bitcast
fast-exp) so the attention phase is no longer scalar-bound. Scores stay
bf16. Residual stream stays f32.
"""

import numpy as np
import ml_dtypes

import concourse.bass as bass
import concourse.bacc as bacc
import concourse.tile as tile
from concourse import mybir
from concourse.bass import ts, ds
from concourse.bass_utils import run_bass_kernel_spmd

f32 = mybir.dt.float32
bf16 = mybir.dt.bfloat16
fp8 = mybir.dt.float8e4
i32 = mybir.dt.int32
AF = mybir.ActivationFunctionType
OP = mybir.AluOpType
PM = mybir.MatmulPerfMode

B, T, C, H = 2, 2048, 1024, 16
DH = C // H          # 64
F = 4 * C            # 4096
NCORES = 8
GROUP = 4            # cores per batch
TQ = T // GROUP      # 512 query rows per core
NT = T // 128        # 16 token tiles
CCH = C // 128       # 8 contraction chunks over C
PAIRS = H // 2       # 8 head pairs
FT = F // 128        # 32 hidden tiles
QT = TQ // 128       # 4 own-row tiles

f16 = mybir.dt.float16

# Schraudolph fast-exp constants: exp(x) ~= bitcast_f32(int(EA*x + EB))
EA = 12102203.161561485   # 2^23 / ln 2
EB = 1064866805.0         # 127*2^23 - 486411 (min-max relative error)
# cidx values whose exp runs on the vector engine (rest on scalar engine)
DVE_EXP = frozenset((2, 6, 10, 13))

_CACHED = {}


def _bcast(ap, parts=128):
    """DRAM AP for a 1-D tensor broadcast across `parts` partitions."""
    return bass.AP(tensor=ap.tensor, offset=ap.offset, ap=[[0, parts]] + list(ap.ap))


def _build_program(trivial_ln1, trivial_ln2, trivial_b):
    nc = bacc.Bacc("TRN2", target_bir_lowering=False, debug=False,
                   num_devices=NCORES)

    xf = nc.dram_tensor("xf", [T, C], bf16, kind="ExternalInput")
    xq = nc.dram_tensor("xq", [TQ, C], f32, kind="ExternalInput")
    # pre-tiled weights: [128 (c within chunk), CCH, out-features] fp8
    wq = nc.dram_tensor("wq", [128, CCH, C], fp8, kind="ExternalInput")
    wk = nc.dram_tensor("wk", [128, CCH, C], fp8, kind="ExternalInput")
    wv = nc.dram_tensor("wv", [128, CCH, C], fp8, kind="ExternalInput")
    bqv = nc.dram_tensor("bq", [128, PAIRS], f32, kind="ExternalInput")
    bkv = nc.dram_tensor("bk", [128, PAIRS], f32, kind="ExternalInput")
    bvv = nc.dram_tensor("bv", [C], f32, kind="ExternalInput")
    ln1w = nc.dram_tensor("ln1w", [C], f32, kind="ExternalInput")
    ln1b = nc.dram_tensor("ln1b", [C], f32, kind="ExternalInput")
    ln2w = nc.dram_tensor("ln2w", [C], f32, kind="ExternalInput")
    ln2b = nc.dram_tensor("ln2b", [C], f32, kind="ExternalInput")
    wp = nc.dram_tensor("wp", [128, CCH, C], fp8, kind="ExternalInput")
    bp = nc.dram_tensor("bp", [C], f32, kind="ExternalInput")
    # wf pre-tiled per f'-tile, split: chunks 0..3 fp8 (DoubleRow), 4..7 f16
    wf8 = nc.dram_tensor("wf8", [FT, 128, CCH // 2, 128], fp8, kind="ExternalInput")
    wff = nc.dram_tensor("wff", [FT, 128, CCH // 2, 128], f16, kind="ExternalInput")
    bf_ = nc.dram_tensor("bf", [128, FT], f32, kind="ExternalInput")
    wm = nc.dram_tensor("wm", [F, C], f16, kind="ExternalInput")
    bm = nc.dram_tensor("bm", [C], f32, kind="ExternalInput")
    out = nc.dram_tensor("out", [TQ, C], f32, kind="ExternalOutput")

    with tile.TileContext(nc) as tc:
        _emit(nc, tc, trivial_ln1, trivial_ln2, trivial_b,
              xf, xq, wq, wk, wv, bqv, bkv, bvv, ln1w, ln1b, ln2w, ln2b,
              wp, bp, wf8, wff, bf_, wm, bm, out)
    nc.compile()
    return nc


def _emit(nc, tc, trivial_ln1, trivial_ln2, trivial_b,
          xf, xq, wq, wk, wv, bqv, bkv, bvv, ln1w, ln1b, ln2w, ln2b,
          wp, bp, wf8, wff, bf_, wm, bm, out):
    from contextlib import ExitStack

    with ExitStack() as st:
        persist = st.enter_context(tc.tile_pool(name="persist", bufs=1))
        stat = st.enter_context(tc.tile_pool(name="stat", bufs=4))
        stream = st.enter_context(tc.tile_pool(name="stream", bufs=5))

        ones64 = persist.tile([1, 64], bf16)
        nc.vector.memset(ones64, 1.0)
        eps_t = persist.tile([128, 1], f32)
        nc.vector.memset(eps_t, 1e-5)

        def layer_norm(x_t, w_bc, b_bc, out_ap, trivial):
            """x_t [128, C] f32 -> out_ap [128, C] bf16 (normalized + affine)."""
            stats = stat.tile([128, 2, nc.vector.BN_STATS_DIM], f32, name="stats", bufs=6)
            nc.vector.bn_stats(out=stats[:, 0, :], in_=x_t[:, 0:512])
            nc.vector.bn_stats(out=stats[:, 1, :], in_=x_t[:, 512:1024])
            mv = stat.tile([128, nc.vector.BN_AGGR_DIM], f32, name="mv", bufs=6)
            nc.vector.bn_aggr(out=mv, in_=stats)
            rstd = stat.tile([128, 1], f32, name="rstd", bufs=6)
            nc.scalar.activation(rstd, mv[:, 1:2], AF.Sqrt, bias=eps_t)
            nc.vector.reciprocal(rstd, rstd)
            if trivial:
                nc.vector.tensor_scalar(out=out_ap, in0=x_t, scalar1=mv[:, 0:1],
                                        scalar2=rstd, op0=OP.subtract, op1=OP.mult)
            else:
                t1 = stat.tile([128, C], f32, name="t1", tag="ln_t1")
                nc.vector.tensor_scalar(out=t1, in0=x_t, scalar1=mv[:, 0:1],
                                        scalar2=rstd, op0=OP.subtract, op1=OP.mult)
                nc.vector.tensor_mul(t1, t1, w_bc)
                nc.vector.tensor_add(out_ap, t1, b_bc)

        # ---------------- pools (stack discipline per side) ----------------
        stA = st.enter_context(ExitStack())
        pA = stA.enter_context(tc.tile_pool(name="pA", bufs=1, side="left"))
        pR = st.enter_context(tc.tile_pool(name="pR", bufs=1, side="right"))
        stB = st.enter_context(ExitStack())
        pB = stB.enter_context(tc.tile_pool(name="pB", bufs=1, side="right"))

        wv_sb = pB.tile([128, CCH, C], fp8)
        if not trivial_b:
            bv_bc = pA.tile([128, C], f32)
            nc.sync.dma_start(out=bv_bc, in_=_bcast(bvv.ap()))
        else:
            bv_bc = None
        if not trivial_ln1:
            ln1w_bc = pA.tile([128, C], f32)
            nc.sync.dma_start(out=ln1w_bc, in_=_bcast(ln1w.ap()))
            ln1b_bc = pA.tile([128, C], f32)
            nc.sync.dma_start(out=ln1b_bc, in_=_bcast(ln1b.ap()))
        else:
            ln1w_bc = ln1b_bc = None

        # chunk-major transposed activations (fp8): [c%128, chunk, tile, token]
        hT8 = pA.tile([128, CCH, NT, 128], fp8)
        # K^T and Q^T for ALL pairs, computed inside p1 (the transpose-DMA
        # wall leaves the PE ~70% idle there; this empties the attention
        # pairs down to scores + PV)
        kT_all = pA.tile([128, PAIRS, T], bf16)
        qT_all = pA.tile([128, PAIRS, TQ], bf16)
        # V (fp8): [key%128, keytile pair, parity, head, DH | ones]
        v8 = pR.tile([128, NT // 2, 2, H, DH + 1], fp8)
        ynT = pR.tile([128, PAIRS, TQ], fp8)
        nc.vector.memset(v8[:, :, :, :, DH:DH + 1], 1.0)

        # ---- LN1 tiles interleaved with V/K/Q matmuls ----
        with nc.named_scope("p1_ln_v"):
            with tc.tile_pool(name="v_ps", bufs=4, space="PSUM") as v_ps, \
                 tc.tile_pool(name="kp_ps", bufs=3, space="PSUM") as kp_ps:
                for i in range(NT):
                    x_t = stream.tile([128, C], bf16, name="x_t", tag="xh_t", bufs=5)
                    nc.sync.dma_start(out=x_t, in_=xf.ap()[ts(i, 128), :])
                    h_t = stream.tile([128, C], bf16, name="h_t", tag="h_t", bufs=5)
                    layer_norm(x_t, ln1w_bc, ln1b_bc, h_t, trivial_ln1)
                    hTb = stream.tile([128, CCH, 128], bf16, name="hTb",
                                      tag="hTb", bufs=5)
                    nc.sync.dma_start_transpose(hTb, h_t[:])
                    # fp8 convert on the scalar engine (idle-ish in this phase)
                    nc.scalar.activation(hT8[:, :, i, :], hTb, AF.Identity)
                    if i == 0:
                        nc.sync.dma_start(out=wv_sb, in_=wv.ap())
                        wk_sb = pA.tile([128, CCH, C], fp8)
                        nc.sync.dma_start(out=wk_sb, in_=wk.ap())
                        bq_sb = pA.tile([128, PAIRS], f32)
                        nc.sync.dma_start(out=bq_sb, in_=bqv.ap())
                        bk_sb = pA.tile([128, PAIRS], f32)
                        nc.sync.dma_start(out=bk_sb, in_=bkv.ap())
                    if i == 8:
                        wq_sb = pA.tile([128, CCH, C], fp8)
                        nc.sync.dma_start(out=wq_sb, in_=wq.ap())
                    pss = [v_ps.tile([128, 512], f32, name=f"ps_v{n}", tag="ps_v")
                           for n in range(2)]
                    for c2 in range(CCH // 2):
                        for n in range(C // 512):
                            nc.tensor.matmul(pss[n], hT8[:, 2 * c2:2 * c2 + 2, i, :],
                                             wv_sb[:, 2 * c2:2 * c2 + 2, ds(512 * n, 512)],
                                             start=(c2 == 0), stop=(c2 == CCH // 2 - 1),
                                             perf_mode=PM.DoubleRow)
                    e_, par = i // 2, i % 2
                    for n in range(C // 512):
                        dst = v8[:, e_, par, 8 * n:8 * n + 8, 0:DH]
                        if trivial_b:
                            # split psum evacuation between scalar and vector
                            if n == 0:
                                nc.scalar.activation(dst, pss[n], AF.Identity)
                            else:
                                nc.vector.tensor_copy(dst, pss[n])
                        else:
                            nc.vector.tensor_add(dst, pss[n],
                                                 bv_bc[:, ds(512 * n, 512)])
                    if i % 4 == 3:
                        # K^T for this 512-token group, all pairs; psum
                        # evacuation alternates scalar/vector
                        g = i // 4
                        for j in range(PAIRS):
                            psn = kp_ps.tile([128, 512], f32, name="ps_k",
                                             tag="ps_k")
                            for c2 in range(CCH // 2):
                                nc.tensor.matmul(
                                    psn, wk_sb[:, 2 * c2:2 * c2 + 2, ts(j, 128)],
                                    hT8[:, 2 * c2:2 * c2 + 2, 4 * g:4 * g + 4, :],
                                    start=(c2 == 0), stop=(c2 == CCH // 2 - 1),
                                    perf_mode=PM.DoubleRow)
                            dst = kT_all[:, j, ds(512 * g, 512)]
                            if j % 2 == 0:
                                nc.scalar.activation(dst, psn, AF.Identity,
                                                     bias=bk_sb[:, j:j + 1])
                            else:
                                nc.vector.tensor_scalar(out=dst, in0=psn,
                                                        scalar1=bk_sb[:, j:j + 1],
                                                        scalar2=None, op0=OP.add)
                        if g == 3:
                            for j in range(PAIRS):
                                psq = kp_ps.tile([128, 512], f32, name="ps_q",
                                                 tag="ps_k")
                                for c2 in range(CCH // 2):
                                    nc.tensor.matmul(
                                        psq, wq_sb[:, 2 * c2:2 * c2 + 2, ts(j, 128)],
                                        hT8[:, 2 * c2:2 * c2 + 2, 0:QT, :],
                                        start=(c2 == 0), stop=(c2 == CCH // 2 - 1),
                                        perf_mode=PM.DoubleRow)
                                dst = qT_all[:, j, :]
                                if j % 2 == 0:
                                    nc.scalar.activation(dst, psq, AF.Identity,
                                                         bias=bq_sb[:, j:j + 1])
                                else:
                                    nc.vector.tensor_scalar(out=dst, in0=psq,
                                                            scalar1=bq_sb[:, j:j + 1],
                                                            scalar2=None, op0=OP.add)
        stB.close()

        # wp prefetch during attention (DMA is idle there)
        wp_sb = pR.tile([128, CCH, C], fp8)
        nc.sync.dma_start(out=wp_sb, in_=wp.ap())
        if not trivial_ln2:
            ln2w_bc = pR.tile([128, C], f32)
            nc.sync.dma_start(out=ln2w_bc, in_=_bcast(ln2w.ap()))
            ln2b_bc = pR.tile([128, C], f32)
            nc.sync.dma_start(out=ln2b_bc, in_=_bcast(ln2b.ap()))
        else:
            ln2w_bc = ln2b_bc = None
        bp_bc = pR.tile([128, C], f32)
        nc.sync.dma_start(out=bp_bc, in_=_bcast(bp.ap()))

        # ---- per-pair attention (K^T/Q^T already computed in p1) ----
        scale = 1.0 / float(np.sqrt(DH))
        with nc.named_scope("p2_attn"), \
             tc.tile_pool(name="s_ps", bufs=2, space="PSUM") as s_ps, \
             tc.tile_pool(name="y_ps", bufs=1, space="PSUM") as y_ps, \
             tc.tile_pool(name="att_sb", bufs=3) as att_sb:
            for j in range(PAIRS):
                kT_j = kT_all[:, j, :]
                qT_j = qT_all[:, j, :]
                ps_y1 = y_ps.tile([DH + 1, 512], f32, name="ps_y1", tag="ps_y1")
                ps_y2 = y_ps.tile([DH + 1, 512], f32, name="ps_y2", tag="ps_y2")
                # software-pipelined with lag 2: PV(cidx-2) is emitted after
                # S(cidx)/exp(cidx) so each exp has ~two matmul slots of
                # latency budget before its PV consumer
                pT_q = []
                for cidx in range(NT):
                    ps_s = s_ps.tile([128, 1024], f32, name="ps_s", tag="ps_s")
                    nc.tensor.matmul(ps_s[:, 0:512],
                                     kT_j[0:64, ts(cidx, 128)],
                                     qT_j[0:64, :], start=True, stop=True)
                    nc.tensor.matmul(ps_s[:, 512:1024],
                                     kT_j[64:128, ts(cidx, 128)],
                                     qT_j[64:128, :], start=True, stop=True,
                                     tile_position=(64, 0))
                    pT = att_sb.tile([128, 2, TQ], bf16, name="pT", tag="pT")
                    if cidx in DVE_EXP:
                        t32 = att_sb.tile([128, 1024], i32, name="t32",
                                          tag="t32", bufs=2)
                        nc.vector.tensor_scalar(out=t32, in0=ps_s,
                                                scalar1=EA * scale, scalar2=EB,
                                                op0=OP.mult, op1=OP.add)
                        nc.vector.tensor_copy(pT, t32[:].bitcast(f32))
                    else:
                        nc.scalar.activation(pT, ps_s, AF.Exp, scale=scale)
                    pT_q.append(pT)
                    if cidx >= 2:
                        pv = cidx - 2
                        for u in range(2):
                            nc.tensor.matmul(ps_y1 if u == 0 else ps_y2,
                                             v8[:, pv // 2, pv % 2, 2 * j + u, :],
                                             pT_q[pv][:, u, :],
                                             start=(pv == 0), stop=False)
                for pv in (NT - 2, NT - 1):
                    for u in range(2):
                        nc.tensor.matmul(ps_y1 if u == 0 else ps_y2,
                                         v8[:, pv // 2, pv % 2, 2 * j + u, :],
                                         pT_q[pv][:, u, :],
                                         start=False, stop=(pv == NT - 1))
                for u, ps_y in ((0, ps_y1), (1, ps_y2)):
                    # copy Y (scalar engine, idle at the pair boundary) and the
                    # sums row out of PSUM right away so the accumulator banks
                    # free up; the sums staging copy also moves them to SBUF
                    # partition 0 (custom-DVE ops mis-read PSUM at an offset)
                    ycp = att_sb.tile([64, 512], f32, name="ycp", tag="ycp")
                    nc.scalar.activation(ycp, ps_y[0:DH, :], AF.Identity)
                    rs0 = att_sb.tile([1, 512], f32, name="rs0", tag="rs0")
                    nc.vector.tensor_copy(rs0, ps_y[DH:DH + 1, :])
                    rs = att_sb.tile([1, 512], f32, name="rs", tag="rs")
                    nc.vector.reciprocal_approx_fast(rs, rs0)
                    rsb = att_sb.tile([1, 512], bf16, name="rsb", tag="rsb")
                    nc.vector.tensor_copy(rsb, rs)
                    # broadcast 1/Z across partitions via a PE outer product
                    # (replaces a serial ~1us gpsimd partition_broadcast)
                    bc = y_ps.tile([64, 512], f32, name="bc", tag="bc_ps",
                                   bufs=2)
                    nc.tensor.matmul(bc, ones64[:], rsb[:], start=True,
                                     stop=True)
                    nc.vector.tensor_mul(ynT[64 * u:64 * u + 64, j, :],
                                         ycp, bc)
        stA.close()

        # ---- attn projection + residual + LN2 + h2^T ----
        pD = st.enter_context(tc.tile_pool(name="pD", bufs=1, side="left"))
        x2 = pD.tile([128, QT, C], f32)
        h2Tb = pD.tile([128, CCH, QT, 128], f16)
        h2T8 = pD.tile([128, CCH // 2, QT, 128], fp8)
        bfc_sb = pD.tile([128, FT], f32)
        nc.sync.dma_start(out=bfc_sb, in_=bf_.ap())
        bm_bc = pD.tile([128, C], f32)
        nc.sync.dma_start(out=bm_bc, in_=_bcast(bm.ap()))

        with nc.named_scope("p3_proj_ln2"):
            with tc.tile_pool(name="ap_ps", bufs=2, space="PSUM") as ap_ps:
                for i in range(QT):
                    xb_t = stream.tile([128, C], f32, name="xb_t", tag="x_t")
                    nc.sync.dma_start(out=xb_t, in_=xq.ap()[ts(i, 128), :])
                    nc.vector.tensor_add(xb_t, xb_t, bp_bc)
                    for n in range(C // 512):
                        ps = ap_ps.tile([128, 512], f32, name="ps_a", tag="ps_a")
                        for a in range(PAIRS // 2):
                            nc.tensor.matmul(ps, ynT[:, 2 * a:2 * a + 2, ts(i, 128)],
                                             wp_sb[:, 2 * a:2 * a + 2, ds(512 * n, 512)],
                                             start=(a == 0), stop=(a == PAIRS // 2 - 1),
                                             perf_mode=PM.DoubleRow)
                        nc.vector.tensor_add(x2[:, i, ds(512 * n, 512)], ps,
                                             xb_t[:, ds(512 * n, 512)])
                    h2_t = stream.tile([128, C], f16, name="h2_t", tag="h2_t", bufs=5)
                    layer_norm(x2[:, i, :], ln2w_bc, ln2b_bc, h2_t, trivial_ln2)
                    nc.sync.dma_start_transpose(h2Tb[:, :, i, :], h2_t[:])
                    nc.scalar.activation(h2T8[:, :, i, :], h2Tb[:, 0:CCH // 2, i, :],
                                         AF.Identity)

        # ---- MLP ----
        # fold the mlp_proj bias into the residual copy while fc runs (DVE idle)
        for i in range(QT):
            nc.vector.tensor_add(x2[:, i, :], x2[:, i, :], bm_bc)
        gTf = pD.tile([128, FT, TQ], f16)
        wm_pool = st.enter_context(tc.tile_pool(name="wm_sb", bufs=5))
        wm_pre = []
        for t in range(2):
            wmt = wm_pool.tile([128, C], f16, name="wm_t", tag="wm_t")
            nc.sync.dma_start(out=wmt, in_=wm.ap()[ts(t, 128), :])
            wm_pre.append(wmt)
        with nc.named_scope("p4_fc"):
            with tc.tile_pool(name="fc_ps", bufs=4, space="PSUM") as fc_ps, \
                 tc.tile_pool(name="wf_sb", bufs=4) as wf_pool:
                for t in range(FT):
                    wf8_t = wf_pool.tile([128, CCH // 2, 128], fp8, name="wf8_t",
                                         tag="wf8_t")
                    nc.sync.dma_start(out=wf8_t, in_=wf8.ap()[t])
                    wff_t = wf_pool.tile([128, CCH // 2, 128], f16, name="wff_t",
                                         tag="wff_t")
                    nc.sync.dma_start(out=wff_t, in_=wff.ap()[t])
                    ps = fc_ps.tile([128, 512], f32, name="ps_f", tag="ps_f")
                    for c2 in range(2):
                        nc.tensor.matmul(ps, wf8_t[:, 2 * c2:2 * c2 + 2, :],
                                         h2T8[:, 2 * c2:2 * c2 + 2, 0:QT, :],
                                         start=(c2 == 0), stop=False,
                                         perf_mode=PM.DoubleRow)
                    for cx in range(CCH // 2):
                        nc.tensor.matmul(ps, wff_t[:, cx, :],
                                         h2Tb[:, CCH // 2 + cx, 0:QT, :],
                                         start=False, stop=(cx == CCH // 2 - 1))
                    nc.scalar.activation(gTf[:, t, :], ps, AF.Gelu_apprx_tanh,
                                         bias=bfc_sb[:, t:t + 1], scale=1.0)

        with nc.named_scope("p5_mlp_out"):
            with tc.tile_pool(name="m_ps", bufs=1, space="PSUM") as m_ps, \
                 tc.tile_pool(name="out_sb", bufs=2) as out_pool:
                ps_m = [m_ps.tile([128, 512], f32, name=f"ps_m{k}", tag=f"ps_m{k}")
                        for k in range(8)]
                for t in range(FT):
                    last = t == FT - 1
                    if t < 2:
                        wm_t = wm_pre[t]
                    else:
                        wm_t = wm_pool.tile([128, C], f16, name="wm_t", tag="wm_t")
                        nc.sync.dma_start(out=wm_t, in_=wm.ap()[ts(t, 128), :])
                    for i in range(QT):
                        for n in range(C // 512):
                            nc.tensor.matmul(ps_m[i * 2 + n],
                                             gTf[:, t, ts(i, 128)],
                                             wm_t[:, ds(512 * n, 512)],
                                             start=(t == 0), stop=last)
                        if last:
                            # drain this i's accumulators immediately so the
                            # final adds + output DMA overlap the remaining MMs
                            out_t = out_pool.tile([128, C], f32, name="out_t",
                                                  tag="out_t")
                            for n in range(C // 512):
                                nc.vector.tensor_add(out_t[:, ds(512 * n, 512)],
                                                     ps_m[i * 2 + n],
                                                     x2[:, i, ds(512 * n, 512)])
                                nc.sync.dma_start(
                                    out=out.ap()[ts(i, 128), ds(512 * n, 512)],
                                    in_=out_t[:, ds(512 * n, 512)])


def _get_program(trivial_ln1, trivial_ln2, trivial_b):
    key = (trivial_ln1, trivial_ln2, trivial_b)
    if key not in _CACHED:
        _CACHED[key] = _build_program(trivial_ln1, trivial_ln2, trivial_b)
    return _CACHED[key]


def _fp8(a):
    return np.ascontiguousarray(np.asarray(a, np.float32)
                                .clip(-240, 240).astype(ml_dtypes.float8_e4m3))


def _tile_proj_weight(w):
    # [C, N] f32 -> [128, CCH, N] fp8 with partition = c % 128, chunk = c // 128
    w = np.asarray(w, np.float32).reshape(CCH, 128, -1)
    return _fp8(w.transpose(1, 0, 2))


def _prep_in_maps(inputs):
    fl = lambda a: np.ascontiguousarray(np.asarray(a, np.float32))
    x = fl(inputs["x"])
    attn_w = fl(inputs["attn_w"])
    attn_b = fl(inputs["attn_b"])
    wf_full = fl(inputs["fc_w"])  # [C, F]
    # wf tiled: [FT, 128(c), CCH, 128(f')]; chunks 0..3 fp8, 4..7 f16
    wf_t = np.ascontiguousarray(
        wf_full.reshape(CCH, 128, FT, 128).transpose(2, 1, 0, 3))
    wf8_t = _fp8(wf_t[:, :, 0:CCH // 2, :])
    wff_t = np.ascontiguousarray(wf_t[:, :, CCH // 2:, :].astype(np.float16))
    wm_t = np.ascontiguousarray(fl(inputs["mlp_proj_w"]).astype(np.float16))
    pb = lambda b: np.ascontiguousarray(
        np.asarray(b, np.float32).reshape(-1, 128).T)  # [128, tiles]
    shared = {
        "wq": _tile_proj_weight(attn_w[:, 0:C]),
        "wk": _tile_proj_weight(attn_w[:, C:2 * C]),
        "wv": _tile_proj_weight(attn_w[:, 2 * C:3 * C]),
        "bq": pb(attn_b[0:C]), "bk": pb(attn_b[C:2 * C]),
        "bv": fl(attn_b[2 * C:3 * C]),
        "ln1w": fl(inputs["ln1_w"]), "ln1b": fl(inputs["ln1_b"]),
        "ln2w": fl(inputs["ln2_w"]), "ln2b": fl(inputs["ln2_b"]),
        "wp": _tile_proj_weight(inputs["attn_proj_w"]),
        "bp": fl(inputs["attn_proj_b"]),
        "wf8": wf8_t, "wff": wff_t, "bf": pb(inputs["fc_b"]),
        "wm": wm_t,
        "bm": fl(inputs["mlp_proj_b"]),
    }
    in_maps = []
    for core in range(NCORES):
        b, r = core // GROUP, core % GROUP
        xb = np.roll(x[b], -TQ * r, axis=0)
        in_maps.append({
            "xf": np.ascontiguousarray(xb.astype(ml_dtypes.bfloat16)),
            "xq": np.ascontiguousarray(xb[0:TQ]),
            **shared,
        })
    return in_maps


def run(inputs, trace=False):
    trivial_ln1 = bool(np.all(np.asarray(inputs["ln1_w"]) == 1.0)
                       and np.all(np.asarray(inputs["ln1_b"]) == 0.0))
    trivial_ln2 = bool(np.all(np.asarray(inputs["ln2_w"]) == 1.0)
                       and np.all(np.asarray(inputs["ln2_b"]) == 0.0))
    trivial_b = bool(np.all(np.asarray(inputs["attn_b"]) == 0.0))
    nc = _get_program(trivial_ln1, trivial_ln2, trivial_b)
    in_maps = _prep_in_maps(inputs)
    res = run_bass_kernel_spmd(nc, in_maps, core_ids=list(range(NCORES)),
                               trace=trace)
    out = np.empty((B, T, C), np.float32)
    for core in range(NCORES):
        b, r = core // GROUP, core % GROUP
        out[b, TQ * r:TQ * (r + 1)] = res.results[core]["out"]
    return out, res


def kernel(**inputs):
    out, _ = run(inputs, trace=False)
    return out



# revision 7
# speedup vs baseline: 1.1314x; 1.1314x over previous
"""Trainium2 Bass kernel for a GPT-2 style transformer block (pre-LN, no mask).

Reference shapes: x [B=2, T=2048, C=1024], H=16 heads, MLP hidden 4C=4096.

Sharding (8 NeuronCores): data-parallel over B (cores 0-3 -> batch 0,
cores 4-7 -> batch 1); within each 4-core group the 2048 query rows are
split 512 per core. Every core redundantly computes K and V for its full
batch from a replicated (rotated) copy of x, so no collectives are needed.

v3 changes vs v2:
  * p1: x is transposed straight from DRAM at t=0 (no LN->transpose
    serialization).  LN1's mean removal is folded into column-centered
    weights host-side; the per-token rstd is broadcast across partitions
    with a PE outer-product and applied in a single fused
    scalar_tensor_tensor that also converts to fp8.  Q is computed as
    soon as the core's own 4 token tiles are ready (not at the end).
  * p2: all softmax probabilities are fp8(e4m3): the scalar engine's Exp
    writes fp8 directly, the vector engine uses a one-op int8 Schraudolph
    exp (bitcast to e4m3), and every P@V matmul runs fp8 DoubleRow.
    1/Z uses a one-op magic-constant bf16 reciprocal.
  * p5: first half of mlp_proj runs fp8 DoubleRow (gelu writes fp8 for
    those tiles directly), second half stays f16.
"""

import numpy as np
import ml_dtypes

import concourse.bass as bass
import concourse.bacc as bacc
import concourse.tile as tile
from concourse import mybir
from concourse.bass import ts, ds
from concourse.bass_utils import run_bass_kernel_spmd

f32 = mybir.dt.float32
bf16 = mybir.dt.bfloat16
f16 = mybir.dt.float16
fp8 = mybir.dt.float8e4
i8 = mybir.dt.int8
i16 = mybir.dt.int16
i32 = mybir.dt.int32
AF = mybir.ActivationFunctionType
OP = mybir.AluOpType
PM = mybir.MatmulPerfMode

B, T, C, H = 2, 2048, 1024, 16
DH = C // H          # 64
F = 4 * C            # 4096
NCORES = 8
GROUP = 4            # cores per batch
TQ = T // GROUP      # 512 query rows per core
NT = T // 128        # 16 token tiles
CCH = C // 128       # 8 contraction chunks over C
PAIRS = H // 2       # 8 head pairs
FT = F // 128        # 32 hidden tiles
QT = TQ // 128       # 4 own-row tiles
FP8T = 8             # mlp_proj tiles done in fp8 DoubleRow (rest f16)

# int8 Schraudolph: e4m3(exp(x)) ~= bitcast_i8(round(EA8*x + EB8))
EA8 = 11.54156033    # 2^3 / ln 2
EB8 = 55.536         # 7*2^3 - 0.0579*2^3 (min-max relative error)
# bf16 magic reciprocal: 1/Z ~= bitcast_bf16(i16(RCP_C - bits_f32(Z)*2^-16))
RCP_C = 32499.0
# cidx values whose exp runs on the scalar engine (rest: DVE int8 fast-exp)
ACT_CIDX = frozenset((0, 2, 4, 6, 8, 10, 12, 14))

_CACHED = {}


def _bcast(ap, parts=128):
    """DRAM AP for a 1-D tensor broadcast across `parts` partitions."""
    return bass.AP(tensor=ap.tensor, offset=ap.offset, ap=[[0, parts]] + list(ap.ap))


def _rep_mid(ap2d, n):
    """[128, W] SBUF AP -> [128, n, W] with stride-0 middle dim."""
    return bass.AP(tensor=ap2d.tensor, offset=ap2d.offset,
                   ap=[list(ap2d.ap[0]), [0, n]] + [list(a) for a in ap2d.ap[1:]])


def _swap12(ap):
    """Swap free dims 1 and 2 of a 4-dim AP (partition, a, b, c) -> (p, b, a, c)."""
    return bass.AP(tensor=ap.tensor, offset=ap.offset,
                   ap=[list(ap.ap[0]), list(ap.ap[2]), list(ap.ap[1]),
                       list(ap.ap[3])])


def _build_program(trivial_b, trivial_ln2):
    nc = bacc.Bacc("TRN2", target_bir_lowering=False, debug=False,
                   num_devices=NCORES)

    xf = nc.dram_tensor("xf", [T, C], bf16, kind="ExternalInput")
    xft = nc.dram_tensor("xft", [NT, 128, CCH, 128], bf16, kind="ExternalInput")
    xq = nc.dram_tensor("xq", [TQ, C], f32, kind="ExternalInput")
    ident = nc.dram_tensor("ident", [128, 128], bf16, kind="ExternalInput")
    # pre-tiled centered weights: [128 (c within chunk), CCH, out-features]
    wq = nc.dram_tensor("wq", [128, CCH, C], fp8, kind="ExternalInput")
    wk = nc.dram_tensor("wk", [128, CCH, C], fp8, kind="ExternalInput")
    wv = nc.dram_tensor("wv", [128, CCH, C], fp8, kind="ExternalInput")
    bqv = nc.dram_tensor("bq", [128, PAIRS], f32, kind="ExternalInput")
    bkv = nc.dram_tensor("bk", [128, PAIRS], f32, kind="ExternalInput")
    bvv = nc.dram_tensor("bv", [C], f32, kind="ExternalInput")
    ln2w = nc.dram_tensor("ln2w", [C], f32, kind="ExternalInput")
    ln2b = nc.dram_tensor("ln2b", [C], f32, kind="ExternalInput")
    wp = nc.dram_tensor("wp", [128, CCH, C], fp8, kind="ExternalInput")
    bp = nc.dram_tensor("bp", [C], f32, kind="ExternalInput")
    # wf pre-tiled per f'-tile, split: chunks 0..3 fp8 (DoubleRow), 4..7 f16
    wf8 = nc.dram_tensor("wf8", [FT, 128, CCH // 2, 128], fp8, kind="ExternalInput")
    wff = nc.dram_tensor("wff", [FT, 128, CCH // 2, 128], f16, kind="ExternalInput")
    bf_ = nc.dram_tensor("bf", [128, FT], f32, kind="ExternalInput")
    # mlp_proj: tiles 0..FP8T-1 as fp8 pairs, rest f16 rows
    wm8 = nc.dram_tensor("wm8", [FP8T // 2, 128, 2, C], fp8, kind="ExternalInput")
    wmf = nc.dram_tensor("wmf", [F - FP8T * 128, C], f16, kind="ExternalInput")
    bm = nc.dram_tensor("bm", [C], f32, kind="ExternalInput")
    out = nc.dram_tensor("out", [TQ, C], f32, kind="ExternalOutput")

    with tile.TileContext(nc) as tc:
        _emit(nc, tc, trivial_b, trivial_ln2,
              xf, xft, xq, ident, wq, wk, wv, bqv, bkv, bvv, ln2w, ln2b,
              wp, bp, wf8, wff, bf_, wm8, wmf, bm, out)
    nc.compile()
    return nc


def _emit(nc, tc, trivial_b, trivial_ln2,
          xf, xft, xq, ident, wq, wk, wv, bqv, bkv, bvv, ln2w, ln2b,
          wp, bp, wf8, wff, bf_, wm8, wmf, bm, out):
    from contextlib import ExitStack

    with ExitStack() as st:
        persist = st.enter_context(tc.tile_pool(name="persist", bufs=1))
        stat = st.enter_context(tc.tile_pool(name="stat", bufs=6))
        stream = st.enter_context(tc.tile_pool(name="stream", bufs=5))

        ones64 = persist.tile([1, 64], bf16)
        nc.vector.memset(ones64, 1.0)
        ones1 = persist.tile([1, 128], bf16)
        nc.vector.memset(ones1, 1.0)
        eps_t = persist.tile([128, 1], f32)
        nc.vector.memset(eps_t, 1e-5)
        ident_sb = persist.tile([128, 128], bf16)
        nc.sync.dma_start(out=ident_sb, in_=ident.ap())

        # ---------------- pools (stack discipline per side) ----------------
        stA = st.enter_context(ExitStack())
        pA = stA.enter_context(tc.tile_pool(name="pA", bufs=1, side="left"))
        pR = st.enter_context(tc.tile_pool(name="pR", bufs=1, side="right"))
        stB = st.enter_context(ExitStack())
        pB = stB.enter_context(tc.tile_pool(name="pB", bufs=1, side="right"))

        # transposed x (host-pretransposed layout, plain contiguous DMA)
        xT_all = pB.tile([128, NT, CCH, 128], bf16)
        # fp8 normalized+scaled activations (tile-major)
        xs8 = pB.tile([128, NT, CCH, 128], fp8)
        bc_all = pB.tile([128, NT, 128], bf16)

        wv_sb = pB.tile([128, CCH, C], fp8)
        wk_sb = pB.tile([128, CCH, C], fp8)
        wq_sb = pB.tile([128, CCH, C], fp8)
        if not trivial_b:
            bv_bc = pB.tile([128, C], f32)
            nc.sync.dma_start(out=bv_bc, in_=_bcast(bvv.ap()))
            bq_sb = pB.tile([128, PAIRS], f32)
            nc.sync.dma_start(out=bq_sb, in_=bqv.ap())
            bk_sb = pB.tile([128, PAIRS], f32)
            nc.sync.dma_start(out=bk_sb, in_=bkv.ap())

        kT_all = pA.tile([128, PAIRS, T], bf16)
        qT_all = pA.tile([128, PAIRS, TQ], bf16)
        v8 = pR.tile([128, NT // 2, 2, H, DH + 1], fp8)
        ynT = pR.tile([128, PAIRS, TQ], fp8)
        nc.vector.memset(v8[:, :, :, :, DH:DH + 1], 1.0)

        def kq_evac(dst, psn, bias_col, which):
            if trivial_b:
                if which % 2 == 0:
                    nc.scalar.activation(dst, psn, AF.Identity)
                else:
                    nc.vector.tensor_copy(dst, psn)
            else:
                if which % 2 == 0:
                    nc.scalar.activation(dst, psn, AF.Identity, bias=bias_col)
                else:
                    nc.vector.tensor_scalar(out=dst, in0=psn, scalar1=bias_col,
                                            scalar2=None, op0=OP.add)

        # ---- p1: stats + rstd-scale-to-fp8 + V/K/Q matmuls ----
        with nc.named_scope("p1_ln_v"):
            with tc.tile_pool(name="v_ps", bufs=4, space="PSUM") as v_ps, \
                 tc.tile_pool(name="kp_ps", bufs=2, space="PSUM") as kp_ps, \
                 tc.tile_pool(name="t_ps", bufs=2, space="PSUM") as t_ps:
                for i in range(NT):
                    x_t = stream.tile([128, C], bf16, name="x_t", tag="x_t", bufs=6)
                    nc.sync.dma_start(out=x_t, in_=xf.ap()[ts(i, 128), :])
                    nc.sync.dma_start(out=xT_all[:, i, :, :], in_=xft.ap()[i])
                    if i == 0:
                        nc.sync.dma_start(out=wv_sb, in_=wv.ap())
                    if i == 1:
                        nc.sync.dma_start(out=wk_sb, in_=wk.ap())
                    if i == 2:
                        nc.sync.dma_start(out=wq_sb, in_=wq.ap())
                    stats = stat.tile([128, 2, nc.vector.BN_STATS_DIM], f32,
                                      name="stats", tag="stats")
                    nc.vector.bn_stats(out=stats[:, 0, :], in_=x_t[:, 0:512])
                    nc.vector.bn_stats(out=stats[:, 1, :], in_=x_t[:, 512:1024])
                    mv = stat.tile([128, nc.vector.BN_AGGR_DIM], f32, name="mv",
                                   tag="mv")
                    nc.vector.bn_aggr(out=mv, in_=stats)
                    rstd_f = stat.tile([128, 1], f32, name="rstd_f", tag="rstd_f")
                    nc.scalar.activation(rstd_f, mv[:, 1:2], AF.Sqrt, bias=eps_t)
                    rstd = stat.tile([128, 1], bf16, name="rstd", tag="rstd")
                    with nc.allow_low_precision(reason="rstd bf16 for PE bcast"):
                        nc.vector.reciprocal(rstd, rstd_f)
                    # broadcast rstd across partitions: transpose + outer product
                    row_ps = t_ps.tile([1, 128], f32, name="row_ps", tag="row_ps", bufs=1)
                    nc.tensor.matmul(row_ps, rstd[:], ident_sb[:], start=True, stop=True)
                    row_sb = stat.tile([1, 128], bf16, name="row_sb", tag="row_sb")
                    nc.scalar.activation(row_sb, row_ps, AF.Identity)
                    bc_ps = t_ps.tile([128, 128], f32, name="bc_ps", tag="bc_ps", bufs=1)
                    nc.tensor.matmul(bc_ps, ones1[:], row_sb[:], start=True, stop=True)
                    nc.scalar.activation(bc_all[:, i, :], bc_ps, AF.Identity)
                    # fused normalize-scale-quantize: xs8 = (xT * rstd_bcast) fp8
                    nc.vector.tensor_mul(xs8[:, i, :, :], xT_all[:, i, :, :],
                                         _rep_mid(bc_all[:, i, :], CCH))

                    # V for this tile
                    pss = [v_ps.tile([128, 512], f32, name=f"ps_v{n}", tag="ps_v")
                           for n in range(2)]
                    for c2 in range(CCH // 2):
                        for n in range(C // 512):
                            nc.tensor.matmul(pss[n], xs8[:, i, 2 * c2:2 * c2 + 2, :],
                                             wv_sb[:, 2 * c2:2 * c2 + 2, ds(512 * n, 512)],
                                             start=(c2 == 0), stop=(c2 == CCH // 2 - 1),
                                             perf_mode=PM.DoubleRow)
                    e_, par = i // 2, i % 2
                    for n in range(C // 512):
                        dst = v8[:, e_, par, 8 * n:8 * n + 8, 0:DH]
                        if trivial_b:
                            if n == 0:
                                nc.scalar.activation(dst, pss[n], AF.Identity)
                            else:
                                nc.vector.tensor_copy(dst, pss[n])
                        else:
                            nc.vector.tensor_add(dst, pss[n],
                                                 bv_bc[:, ds(512 * n, 512)])

                    if i == 3:
                        # Q for the core's own rows (tiles 0..3) -- early
                        for j in range(PAIRS):
                            psq = kp_ps.tile([128, 512], f32, name="ps_q",
                                             tag="ps_k")
                            for c2 in range(CCH // 2):
                                rhs = _swap12(xs8[:, 0:4, 2 * c2:2 * c2 + 2, :])
                                nc.tensor.matmul(
                                    psq, wq_sb[:, 2 * c2:2 * c2 + 2, ts(j, 128)],
                                    rhs, start=(c2 == 0),
                                    stop=(c2 == CCH // 2 - 1),
                                    perf_mode=PM.DoubleRow)
                            kq_evac(qT_all[:, j, :], psq,
                                    None if trivial_b else bq_sb[:, j:j + 1], j)
                    if i % 4 == 3:
                        g = i // 4
                        for j in range(PAIRS):
                            psn = kp_ps.tile([128, 512], f32, name="ps_k",
                                             tag="ps_k")
                            for c2 in range(CCH // 2):
                                rhs = _swap12(
                                    xs8[:, 4 * g:4 * g + 4, 2 * c2:2 * c2 + 2, :])
                                nc.tensor.matmul(
                                    psn, wk_sb[:, 2 * c2:2 * c2 + 2, ts(j, 128)],
                                    rhs, start=(c2 == 0),
                                    stop=(c2 == CCH // 2 - 1),
                                    perf_mode=PM.DoubleRow)
                            kq_evac(kT_all[:, j, ds(512 * g, 512)], psn,
                                    None if trivial_b else bk_sb[:, j:j + 1], j + 1)
        stB.close()

        # wp + p3 constants prefetch during attention (DMA is idle there)
        wp_sb = pR.tile([128, CCH, C], fp8)
        nc.sync.dma_start(out=wp_sb, in_=wp.ap())
        xq_sb = pR.tile([128, QT, C], f32)
        for i in range(QT):
            nc.sync.dma_start(out=xq_sb[:, i, :], in_=xq.ap()[ts(i, 128), :])
        ln2w_bc = pR.tile([128, C], f32)
        nc.sync.dma_start(out=ln2w_bc, in_=_bcast(ln2w.ap()))
        ln2b_bc = pR.tile([128, C], f32)
        nc.sync.dma_start(out=ln2b_bc, in_=_bcast(ln2b.ap()))
        bp_bc = pR.tile([128, C], f32)
        nc.sync.dma_start(out=bp_bc, in_=_bcast(bp.ap()))

        # ---- p2: per-pair attention; all probabilities fp8, all P@V DR ----
        scale = 1.0 / float(np.sqrt(DH))
        with nc.named_scope("p2_attn"), \
             tc.tile_pool(name="s_ps", bufs=3, space="PSUM") as s_ps, \
             tc.tile_pool(name="y_ps", bufs=1, space="PSUM") as y_ps, \
             tc.tile_pool(name="att_sb", bufs=3) as att_sb:
            for j in range(PAIRS):
                kT_j = kT_all[:, j, :]
                qT_j = qT_all[:, j, :]
                ps_y1 = y_ps.tile([DH + 1, 512], f32, name="ps_y1", tag="ps_y1")
                ps_y2 = y_ps.tile([DH + 1, 512], f32, name="ps_y2", tag="ps_y2")
                p8s = []

                def pv(e):
                    p8 = p8s[e]
                    for u in range(2):
                        nc.tensor.matmul(ps_y1 if u == 0 else ps_y2,
                                         v8[:, e, :, 2 * j + u, :],
                                         p8[:, :, u, :].bitcast(fp8),
                                         start=(e == 0), stop=(e == NT // 2 - 1),
                                         perf_mode=PM.DoubleRow)

                for e in range(NT // 2):
                    p8 = att_sb.tile([128, 2, 2, 512], i8, name="p8", tag="p8",
                                     bufs=3)
                    p8s.append(p8)
                    for par in range(2):
                        cidx = 2 * e + par
                        ps_s = s_ps.tile([128, 1024], f32, name="ps_s", tag="ps_s")
                        nc.tensor.matmul(ps_s[:, 0:512],
                                         kT_j[0:64, ts(cidx, 128)],
                                         qT_j[0:64, :], start=True, stop=True)
                        nc.tensor.matmul(ps_s[:, 512:1024],
                                         kT_j[64:128, ts(cidx, 128)],
                                         qT_j[64:128, :], start=True, stop=True,
                                         tile_position=(64, 0))
                        dst = p8[:, par, :, :]
                        if cidx in ACT_CIDX:
                            nc.scalar.activation(dst.bitcast(fp8), ps_s, AF.Exp,
                                                 scale=scale)
                        else:
                            nc.vector.tensor_scalar(out=dst, in0=ps_s,
                                                    scalar1=EA8 * scale,
                                                    scalar2=EB8,
                                                    op0=OP.mult, op1=OP.add)
                    if e >= 1:
                        pv(e - 1)
                pv(NT // 2 - 1)

                for u, ps_y in ((0, ps_y1), (1, ps_y2)):
                    # y out of PSUM (frees accumulator bank); 1/Z via a one-op
                    # magic-constant bf16 reciprocal; broadcast via PE outer
                    ycp = att_sb.tile([64, 512], f32, name="ycp", tag="ycp")
                    if u == 0:
                        nc.scalar.activation(ycp, ps_y[0:DH, :], AF.Identity)
                    else:
                        nc.vector.tensor_copy(ycp, ps_y[0:DH, :])
                    rsb = att_sb.tile([1, 512], i16, name="rsb", tag="rsb")
                    nc.vector.tensor_scalar(
                        out=rsb, in0=ps_y[DH:DH + 1, :].bitcast(i32),
                        scalar1=-(2.0 ** -16), scalar2=RCP_C,
                        op0=OP.mult, op1=OP.add)
                    # 1/Z broadcast reuses the drained y accumulator bank
                    bc = ps_y[0:DH, :]
                    nc.tensor.matmul(bc, ones64[:], rsb[:].bitcast(bf16),
                                     start=True, stop=True)
                    nc.vector.tensor_mul(ynT[64 * u:64 * u + 64, j, :],
                                         ycp, bc)
        stA.close()

        # ---- p3: attn projection + residual + LN2 + h2^T ----
        pD = st.enter_context(tc.tile_pool(name="pD", bufs=1, side="left"))
        x2 = pD.tile([128, QT, C], f32)
        h2Tb = pD.tile([128, CCH, QT, 128], f16)
        h2T8 = pD.tile([128, CCH // 2, QT, 128], fp8)
        bfc_sb = pD.tile([128, FT], f32)
        nc.sync.dma_start(out=bfc_sb, in_=bf_.ap())
        bm_bc = pD.tile([128, C], f32)
        nc.sync.dma_start(out=bm_bc, in_=_bcast(bm.ap()))

        def layer_norm2(x_t, out_ap):
            stats = stat.tile([128, 2, nc.vector.BN_STATS_DIM], f32,
                              name="stats2", tag="stats")
            nc.vector.bn_stats(out=stats[:, 0, :], in_=x_t[:, 0:512])
            nc.vector.bn_stats(out=stats[:, 1, :], in_=x_t[:, 512:1024])
            mv = stat.tile([128, nc.vector.BN_AGGR_DIM], f32, name="mv2",
                           tag="mv")
            nc.vector.bn_aggr(out=mv, in_=stats)
            rstd = stat.tile([128, 1], f32, name="rstd2", tag="rstd_f")
            nc.scalar.activation(rstd, mv[:, 1:2], AF.Sqrt, bias=eps_t)
            nc.vector.reciprocal(rstd, rstd)
            if trivial_ln2:
                nc.vector.tensor_scalar(out=out_ap, in0=x_t, scalar1=mv[:, 0:1],
                                        scalar2=rstd, op0=OP.subtract,
                                        op1=OP.mult)
            else:
                t1 = stat.tile([128, C], f32, name="t1", tag="ln_t1")
                nc.vector.tensor_scalar(out=t1, in0=x_t, scalar1=mv[:, 0:1],
                                        scalar2=rstd, op0=OP.subtract,
                                        op1=OP.mult)
                nc.vector.tensor_mul(t1, t1, ln2w_bc)
                nc.vector.tensor_add(out_ap, t1, ln2b_bc)

        with nc.named_scope("p3_proj_ln2"):
            with tc.tile_pool(name="ap_ps", bufs=2, space="PSUM") as ap_ps:
                h2_ts = []
                for i in range(QT):
                    xb_t = xq_sb[:, i, :]
                    nc.vector.tensor_add(xb_t, xb_t, bp_bc)
                    for n in range(C // 512):
                        ps = ap_ps.tile([128, 512], f32, name="ps_a", tag="ps_a")
                        for a in range(PAIRS // 2):
                            nc.tensor.matmul(ps, ynT[:, 2 * a:2 * a + 2, ts(i, 128)],
                                             wp_sb[:, 2 * a:2 * a + 2, ds(512 * n, 512)],
                                             start=(a == 0), stop=(a == PAIRS // 2 - 1),
                                             perf_mode=PM.DoubleRow)
                        nc.vector.tensor_add(x2[:, i, ds(512 * n, 512)], ps,
                                             xb_t[:, ds(512 * n, 512)])
                    h2_t = stream.tile([128, C], f16, name="h2_t", tag="h2_t", bufs=5)
                    layer_norm2(x2[:, i, :], h2_t)
                    h2_ts.append(h2_t)
                # batch the SBUF->SBUF transposes (they exclude other DMA
                # traffic; batched they share one exclusion window)
                for i in range(QT):
                    nc.scalar.dma_start_transpose(h2Tb[:, :, i, :], h2_ts[i][:])
                    nc.scalar.activation(h2T8[:, :, i, :], h2Tb[:, 0:CCH // 2, i, :],
                                         AF.Identity)

        # ---- p4: MLP fc + gelu ----
        # fold the mlp_proj bias into the residual copy while fc runs (DVE idle)
        for i in range(QT):
            nc.vector.tensor_add(x2[:, i, :], x2[:, i, :], bm_bc)
        gTf8 = pD.tile([128, FP8T, TQ], fp8)
        gTff = pD.tile([128, FT - FP8T, TQ], f16)
        wm_pool = st.enter_context(tc.tile_pool(name="wm_sb", bufs=5))
        with nc.named_scope("p4_fc"):
            with tc.tile_pool(name="fc_ps", bufs=4, space="PSUM") as fc_ps, \
                 tc.tile_pool(name="wf_sb", bufs=4) as wf_pool:
                for t in range(FT):
                    wf8_t = wf_pool.tile([128, CCH // 2, 128], fp8, name="wf8_t",
                                         tag="wf8_t")
                    nc.sync.dma_start(out=wf8_t, in_=wf8.ap()[t])
                    wff_t = wf_pool.tile([128, CCH // 2, 128], f16, name="wff_t",
                                         tag="wff_t")
                    nc.sync.dma_start(out=wff_t, in_=wff.ap()[t])
                    ps = fc_ps.tile([128, 512], f32, name="ps_f", tag="ps_f")
                    for c2 in range(2):
                        nc.tensor.matmul(ps, wf8_t[:, 2 * c2:2 * c2 + 2, :],
                                         h2T8[:, 2 * c2:2 * c2 + 2, 0:QT, :],
                                         start=(c2 == 0), stop=False,
                                         perf_mode=PM.DoubleRow)
                    for cx in range(CCH // 2):
                        nc.tensor.matmul(ps, wff_t[:, cx, :],
                                         h2Tb[:, CCH // 2 + cx, 0:QT, :],
                                         start=False, stop=(cx == CCH // 2 - 1))
                    if t < FP8T:
                        nc.scalar.activation(gTf8[:, t, :], ps, AF.Gelu_apprx_tanh,
                                             bias=bfc_sb[:, t:t + 1], scale=1.0)
                    else:
                        nc.scalar.activation(gTff[:, t - FP8T, :], ps,
                                             AF.Gelu_apprx_tanh,
                                             bias=bfc_sb[:, t:t + 1], scale=1.0)

        # ---- p5: MLP out projection (half fp8 DoubleRow, half f16) ----
        with nc.named_scope("p5_mlp_out"):
            with tc.tile_pool(name="m_ps", bufs=1, space="PSUM") as m_ps, \
                 tc.tile_pool(name="out_sb", bufs=2) as out_pool:
                ps_m = [m_ps.tile([128, 512], f32, name=f"ps_m{k}", tag=f"ps_m{k}")
                        for k in range(8)]
                for a in range(FP8T // 2):
                    wm_t = wm_pool.tile([128, 2, C], fp8, name="wm8_t", tag="wm8_t")
                    nc.sync.dma_start(out=wm_t, in_=wm8.ap()[a])
                    for i in range(QT):
                        for n in range(C // 512):
                            nc.tensor.matmul(ps_m[i * 2 + n],
                                             gTf8[:, 2 * a:2 * a + 2, ts(i, 128)],
                                             wm_t[:, :, ds(512 * n, 512)],
                                             start=(a == 0), stop=False,
                                             perf_mode=PM.DoubleRow)
                for t in range(FT - FP8T):
                    last = t == FT - FP8T - 1
                    wm_t = wm_pool.tile([128, C], f16, name="wmf_t", tag="wmf_t")
                    nc.sync.dma_start(out=wm_t, in_=wmf.ap()[ts(t, 128), :])
                    for i in range(QT):
                        for n in range(C // 512):
                            nc.tensor.matmul(ps_m[i * 2 + n],
                                             gTff[:, t, ts(i, 128)],
                                             wm_t[:, ds(512 * n, 512)],
                                             start=False, stop=last)
                        if last:
                            # drain this i's accumulators immediately so the
                            # final adds + output DMA overlap the remaining MMs
                            out_t = out_pool.tile([128, C], f32, name="out_t",
                                                  tag="out_t")
                            for n in range(C // 512):
                                nc.vector.tensor_add(out_t[:, ds(512 * n, 512)],
                                                     ps_m[i * 2 + n],
                                                     x2[:, i, ds(512 * n, 512)])
                                nc.sync.dma_start(
                                    out=out.ap()[ts(i, 128), ds(512 * n, 512)],
                                    in_=out_t[:, ds(512 * n, 512)])


def _get_program(trivial_b, trivial_ln2):
    key = (trivial_b, trivial_ln2)
    if key not in _CACHED:
        _CACHED[key] = _build_program(trivial_b, trivial_ln2)
    return _CACHED[key]


def _fp8(a):
    return np.ascontiguousarray(np.asarray(a, np.float32)
                                .clip(-240, 240).astype(ml_dtypes.float8_e4m3))


def _tile_proj_weight(w):
    # [C, N] f32 -> [128, CCH, N] fp8 with partition = c % 128, chunk = c // 128
    w = np.asarray(w, np.float32).reshape(CCH, 128, -1)
    return _fp8(w.transpose(1, 0, 2))


def _prep_in_maps(inputs):
    fl = lambda a: np.ascontiguousarray(np.asarray(a, np.float32))
    x = fl(inputs["x"])
    ln1w = fl(inputs["ln1_w"])
    ln1b = fl(inputs["ln1_b"])
    attn_w = fl(inputs["attn_w"]) * ln1w[:, None]      # fold LN1 gamma
    battn = ln1b @ attn_w + fl(inputs["attn_b"])       # fold LN1 beta
    attn_w = attn_w - attn_w.mean(0, keepdims=True)    # fold mean removal
    wf_full = fl(inputs["fc_w"])  # [C, F]
    # wf tiled: [FT, 128(c), CCH, 128(f')]; chunks 0..3 fp8, 4..7 f16
    wf_t = np.ascontiguousarray(
        wf_full.reshape(CCH, 128, FT, 128).transpose(2, 1, 0, 3))
    wf8_t = _fp8(wf_t[:, :, 0:CCH // 2, :])
    wff_t = np.ascontiguousarray(wf_t[:, :, CCH // 2:, :].astype(np.float16))
    wm_full = fl(inputs["mlp_proj_w"])  # [F, C]
    wm8_t = _fp8(wm_full[0:FP8T * 128].reshape(FP8T // 2, 2, 128, C)
                 .transpose(0, 2, 1, 3))
    wmf_t = np.ascontiguousarray(wm_full[FP8T * 128:].astype(np.float16))
    pb = lambda b: np.ascontiguousarray(
        np.asarray(b, np.float32).reshape(-1, 128).T)  # [128, tiles]
    shared = {
        "ident": np.ascontiguousarray(np.eye(128, dtype=np.float32)
                                      .astype(ml_dtypes.bfloat16)),
        "wq": _tile_proj_weight(attn_w[:, 0:C]),
        "wk": _tile_proj_weight(attn_w[:, C:2 * C]),
        "wv": _tile_proj_weight(attn_w[:, 2 * C:3 * C]),
        "bq": pb(battn[0:C]), "bk": pb(battn[C:2 * C]),
        "bv": fl(battn[2 * C:3 * C]),
        "ln2w": fl(inputs["ln2_w"]), "ln2b": fl(inputs["ln2_b"]),
        "wp": _tile_proj_weight(inputs["attn_proj_w"]),
        "bp": fl(inputs["attn_proj_b"]),
        "wf8": wf8_t, "wff": wff_t, "bf": pb(inputs["fc_b"]),
        "wm8": wm8_t, "wmf": wmf_t,
        "bm": fl(inputs["mlp_proj_b"]),
    }
    in_maps = []
    for core in range(NCORES):
        b, r = core // GROUP, core % GROUP
        xb = np.roll(x[b], -TQ * r, axis=0)
        xb16 = xb.astype(ml_dtypes.bfloat16)
        in_maps.append({
            "xf": np.ascontiguousarray(xb16),
            "xft": np.ascontiguousarray(
                xb16.reshape(NT, 128, CCH, 128).transpose(0, 3, 2, 1)),
            "xq": np.ascontiguousarray(xb[0:TQ]),
            **shared,
        })
    return in_maps


def run(inputs, trace=False):
    fl = lambda a: np.asarray(a, np.float32)
    battn = fl(inputs["ln1_b"]) @ (fl(inputs["attn_w"])
                                   * fl(inputs["ln1_w"])[:, None]) \
        + fl(inputs["attn_b"])
    trivial_b = bool(np.all(np.abs(battn) < 1e-12))
    trivial_ln2 = bool(np.all(np.asarray(inputs["ln2_w"]) == 1.0)
                       and np.all(np.asarray(inputs["ln2_b"]) == 0.0))
    nc = _get_program(trivial_b, trivial_ln2)
    in_maps = _prep_in_maps(inputs)
    res = run_bass_kernel_spmd(nc, in_maps, core_ids=list(range(NCORES)),
                               trace=trace)
    out = np.empty((B, T, C), np.float32)
    for core in range(NCORES):
        b, r = core // GROUP, core % GROUP
        out[b, TQ * r:TQ * (r + 1)] = res.results[core]["out"]
    return out, res


def kernel(**inputs):
    out, _ = run(inputs, trace=False)
    return out


# revision 8
# speedup vs baseline: 1.1493x; 1.0158x over previous
"""Trainium2 Bass kernel for a GPT-2 style transformer block (pre-LN, no mask).

Reference shapes: x [B=2, T=2048, C=1024], H=16 heads, MLP hidden 4C=4096.

Sharding (8 NeuronCores): data-parallel over B (cores 0-3 -> batch 0,
cores 4-7 -> batch 1); within each 4-core group the 2048 query rows are
split 512 per core. Every core redundantly computes K and V for its full
batch from a replicated (rotated) copy of x, so no collectives are needed.

v3 changes vs v2:
  * p1: x is transposed straight from DRAM at t=0 (no LN->transpose
    serialization).  LN1's mean removal is folded into column-centered
    weights host-side; the per-token rstd is broadcast across partitions
    with a PE outer-product and applied in a single fused
    scalar_tensor_tensor that also converts to fp8.  Q is computed as
    soon as the core's own 4 token tiles are ready (not at the end).
  * p2: all softmax probabilities are fp8(e4m3): the scalar engine's Exp
    writes fp8 directly, the vector engine uses a one-op int8 Schraudolph
    exp (bitcast to e4m3), and every P@V matmul runs fp8 DoubleRow.
    1/Z uses a one-op magic-constant bf16 reciprocal.
  * p5: first half of mlp_proj runs fp8 DoubleRow (gelu writes fp8 for
    those tiles directly), second half stays f16.
"""

import numpy as np
import ml_dtypes

import concourse.bass as bass
import concourse.bacc as bacc
import concourse.tile as tile
from concourse import mybir
from concourse.bass import ts, ds
from concourse.bass_utils import run_bass_kernel_spmd

f32 = mybir.dt.float32
bf16 = mybir.dt.bfloat16
f16 = mybir.dt.float16
fp8 = mybir.dt.float8e4
i8 = mybir.dt.int8
i16 = mybir.dt.int16
i32 = mybir.dt.int32
AF = mybir.ActivationFunctionType
OP = mybir.AluOpType
PM = mybir.MatmulPerfMode

B, T, C, H = 2, 2048, 1024, 16
DH = C // H          # 64
F = 4 * C            # 4096
NCORES = 8
GROUP = 4            # cores per batch
TQ = T // GROUP      # 512 query rows per core
NT = T // 128        # 16 token tiles
CCH = C // 128       # 8 contraction chunks over C
PAIRS = H // 2       # 8 head pairs
FT = F // 128        # 32 hidden tiles
QT = TQ // 128       # 4 own-row tiles
FP8T = 8             # mlp_proj tiles done in fp8 DoubleRow (rest f16)

# int8 Schraudolph: e4m3(exp(x)) ~= bitcast_i8(round(EA8*x + EB8))
EA8 = 11.54156033    # 2^3 / ln 2
EB8 = 55.536         # 7*2^3 - 0.0579*2^3 (min-max relative error)
# bf16 magic reciprocal: 1/Z ~= bitcast_bf16(i16(RCP_C - bits_f32(Z)*2^-16))
RCP_C = 32499.0
# cidx values whose exp runs on the scalar engine (rest: DVE int8 fast-exp)
ACT_CIDX = frozenset((0, 2, 4, 6, 8, 10, 12, 14, 15))

_CACHED = {}


def _bcast(ap, parts=128):
    """DRAM AP for a 1-D tensor broadcast across `parts` partitions."""
    return bass.AP(tensor=ap.tensor, offset=ap.offset, ap=[[0, parts]] + list(ap.ap))


def _rep_mid(ap2d, n):
    """[128, W] SBUF AP -> [128, n, W] with stride-0 middle dim."""
    return bass.AP(tensor=ap2d.tensor, offset=ap2d.offset,
                   ap=[list(ap2d.ap[0]), [0, n]] + [list(a) for a in ap2d.ap[1:]])


def _swap12(ap):
    """Swap free dims 1 and 2 of a 4-dim AP (partition, a, b, c) -> (p, b, a, c)."""
    return bass.AP(tensor=ap.tensor, offset=ap.offset,
                   ap=[list(ap.ap[0]), list(ap.ap[2]), list(ap.ap[1]),
                       list(ap.ap[3])])


def _build_program(trivial_b, trivial_ln2):
    nc = bacc.Bacc("TRN2", target_bir_lowering=False, debug=False,
                   num_devices=NCORES)

    xf = nc.dram_tensor("xf", [T, C], bf16, kind="ExternalInput")
    xft = nc.dram_tensor("xft", [NT, 128, CCH, 128], bf16, kind="ExternalInput")
    xq = nc.dram_tensor("xq", [TQ, C], f32, kind="ExternalInput")
    ident = nc.dram_tensor("ident", [128, 128], bf16, kind="ExternalInput")
    # pre-tiled centered weights: [128 (c within chunk), CCH, out-features]
    wq = nc.dram_tensor("wq", [128, CCH, C], fp8, kind="ExternalInput")
    wk = nc.dram_tensor("wk", [128, CCH, C], fp8, kind="ExternalInput")
    wv = nc.dram_tensor("wv", [128, CCH, C], fp8, kind="ExternalInput")
    bqv = nc.dram_tensor("bq", [128, PAIRS], f32, kind="ExternalInput")
    bkv = nc.dram_tensor("bk", [128, PAIRS], f32, kind="ExternalInput")
    bvv = nc.dram_tensor("bv", [C], f32, kind="ExternalInput")
    ln2w = nc.dram_tensor("ln2w", [C], f32, kind="ExternalInput")
    ln2b = nc.dram_tensor("ln2b", [C], f32, kind="ExternalInput")
    wp = nc.dram_tensor("wp", [128, CCH, C], fp8, kind="ExternalInput")
    bp = nc.dram_tensor("bp", [C], f32, kind="ExternalInput")
    # wf pre-tiled per f'-tile, split: chunks 0..3 fp8 (DoubleRow), 4..7 f16
    wf8 = nc.dram_tensor("wf8", [FT, 128, CCH // 2, 128], fp8, kind="ExternalInput")
    wff = nc.dram_tensor("wff", [FT, 128, CCH // 2, 128], f16, kind="ExternalInput")
    bf_ = nc.dram_tensor("bf", [128, FT], f32, kind="ExternalInput")
    # mlp_proj: tiles 0..FP8T-1 as fp8 pairs, rest f16 rows
    wm8 = nc.dram_tensor("wm8", [FP8T // 2, 128, 2, C], fp8, kind="ExternalInput")
    wmf = nc.dram_tensor("wmf", [F - FP8T * 128, C], f16, kind="ExternalInput")
    bm = nc.dram_tensor("bm", [C], f32, kind="ExternalInput")
    out = nc.dram_tensor("out", [TQ, C], f32, kind="ExternalOutput")

    with tile.TileContext(nc) as tc:
        _emit(nc, tc, trivial_b, trivial_ln2,
              xf, xft, xq, ident, wq, wk, wv, bqv, bkv, bvv, ln2w, ln2b,
              wp, bp, wf8, wff, bf_, wm8, wmf, bm, out)
    nc.compile()
    return nc


def _emit(nc, tc, trivial_b, trivial_ln2,
          xf, xft, xq, ident, wq, wk, wv, bqv, bkv, bvv, ln2w, ln2b,
          wp, bp, wf8, wff, bf_, wm8, wmf, bm, out):
    from contextlib import ExitStack

    with ExitStack() as st:
        persist = st.enter_context(tc.tile_pool(name="persist", bufs=1))
        stat = st.enter_context(tc.tile_pool(name="stat", bufs=6))
        stream = st.enter_context(tc.tile_pool(name="stream", bufs=5))

        ones64 = persist.tile([1, 64], bf16)
        nc.vector.memset(ones64, 1.0)
        ones1 = persist.tile([1, 128], bf16)
        nc.vector.memset(ones1, 1.0)
        eps_t = persist.tile([128, 1], f32)
        nc.vector.memset(eps_t, 1e-5)
        ident_sb = persist.tile([128, 128], bf16)
        nc.sync.dma_start(out=ident_sb, in_=ident.ap())

        # ---------------- pools (stack discipline per side) ----------------
        stA = st.enter_context(ExitStack())
        pA = stA.enter_context(tc.tile_pool(name="pA", bufs=1, side="left"))
        stC = st.enter_context(ExitStack())
        pR = stC.enter_context(tc.tile_pool(name="pR", bufs=1, side="right"))
        stB = st.enter_context(ExitStack())
        pB = stB.enter_context(tc.tile_pool(name="pB", bufs=1, side="right"))

        # transposed x (host-pretransposed layout, plain contiguous DMA)
        xT_all = pB.tile([128, NT, CCH, 128], bf16)
        # fp8 normalized+scaled activations (tile-major)
        xs8 = pB.tile([128, NT, CCH, 128], fp8)
        bc_all = pB.tile([128, NT, 128], bf16)

        wv_sb = pB.tile([128, CCH, C], fp8)
        wk_sb = pB.tile([128, CCH, C], fp8)
        wq_sb = pB.tile([128, CCH, C], fp8)
        if not trivial_b:
            bv_bc = pB.tile([128, C], f32)
            nc.sync.dma_start(out=bv_bc, in_=_bcast(bvv.ap()))
            bq_sb = pB.tile([128, PAIRS], f32)
            nc.sync.dma_start(out=bq_sb, in_=bqv.ap())
            bk_sb = pB.tile([128, PAIRS], f32)
            nc.sync.dma_start(out=bk_sb, in_=bkv.ap())

        kT_all = pA.tile([128, PAIRS, T], bf16)
        qT_all = pA.tile([128, PAIRS, TQ], bf16)
        v8 = pR.tile([128, NT // 2, 2, H, DH + 1], fp8)
        ynT = pR.tile([128, PAIRS, TQ], fp8)
        nc.vector.memset(v8[:, :, :, :, DH:DH + 1], 1.0)

        def kq_evac(dst, psn, bias_col, which):
            if trivial_b:
                nc.scalar.activation(dst, psn, AF.Identity)
            else:
                nc.scalar.activation(dst, psn, AF.Identity, bias=bias_col)

        # ---- p1: stats + rstd-scale-to-fp8 + V/K/Q matmuls ----
        with nc.named_scope("p1_ln_v"):
            with tc.tile_pool(name="v_ps", bufs=4, space="PSUM") as v_ps, \
                 tc.tile_pool(name="kp_ps", bufs=2, space="PSUM") as kp_ps, \
                 tc.tile_pool(name="t_ps", bufs=2, space="PSUM") as t_ps:
                for i in range(NT):
                    x_t = stream.tile([128, C], bf16, name="x_t", tag="x_t", bufs=6)
                    nc.sync.dma_start(out=x_t, in_=xf.ap()[ts(i, 128), :])
                    nc.sync.dma_start(out=xT_all[:, i, :, :], in_=xft.ap()[i])
                    if i == 0:
                        nc.sync.dma_start(out=wv_sb, in_=wv.ap())
                    if i == 1:
                        nc.sync.dma_start(out=wk_sb, in_=wk.ap())
                    if i == 2:
                        nc.sync.dma_start(out=wq_sb, in_=wq.ap())
                    stats = stat.tile([128, 2, nc.vector.BN_STATS_DIM], f32,
                                      name="stats", tag="stats")
                    nc.vector.bn_stats(out=stats[:, 0, :], in_=x_t[:, 0:512])
                    nc.vector.bn_stats(out=stats[:, 1, :], in_=x_t[:, 512:1024])
                    mv = stat.tile([128, nc.vector.BN_AGGR_DIM], f32, name="mv",
                                   tag="mv")
                    nc.vector.bn_aggr(out=mv, in_=stats)
                    rstd_f = stat.tile([128, 1], f32, name="rstd_f", tag="rstd_f")
                    nc.scalar.activation(rstd_f, mv[:, 1:2], AF.Sqrt, bias=eps_t)
                    rstd = stat.tile([128, 1], bf16, name="rstd", tag="rstd")
                    with nc.allow_low_precision(reason="rstd bf16 for PE bcast"):
                        nc.vector.reciprocal(rstd, rstd_f)
                    # broadcast rstd across partitions: transpose + outer product
                    row_ps = t_ps.tile([1, 128], f32, name="row_ps", tag="row_ps", bufs=1)
                    nc.tensor.matmul(row_ps, rstd[:], ident_sb[:], start=True, stop=True)
                    row_sb = stat.tile([1, 128], bf16, name="row_sb", tag="row_sb")
                    nc.scalar.activation(row_sb, row_ps, AF.Identity)
                    bc_ps = t_ps.tile([128, 128], f32, name="bc_ps", tag="bc_ps", bufs=1)
                    nc.tensor.matmul(bc_ps, ones1[:], row_sb[:], start=True, stop=True)
                    nc.scalar.activation(bc_all[:, i, :], bc_ps, AF.Identity)
                    # fused normalize-scale-quantize: xs8 = (xT * rstd_bcast) fp8
                    nc.vector.tensor_mul(xs8[:, i, :, :], xT_all[:, i, :, :],
                                         _rep_mid(bc_all[:, i, :], CCH))

                    # V for this tile
                    pss = [v_ps.tile([128, 512], f32, name=f"ps_v{n}", tag="ps_v")
                           for n in range(2)]
                    for c2 in range(CCH // 2):
                        for n in range(C // 512):
                            nc.tensor.matmul(pss[n], xs8[:, i, 2 * c2:2 * c2 + 2, :],
                                             wv_sb[:, 2 * c2:2 * c2 + 2, ds(512 * n, 512)],
                                             start=(c2 == 0), stop=(c2 == CCH // 2 - 1),
                                             perf_mode=PM.DoubleRow)
                    e_, par = i // 2, i % 2
                    for n in range(C // 512):
                        dst = v8[:, e_, par, 8 * n:8 * n + 8, 0:DH]
                        if trivial_b:
                            nc.scalar.activation(dst, pss[n], AF.Identity)
                        else:
                            nc.vector.tensor_add(dst, pss[n],
                                                 bv_bc[:, ds(512 * n, 512)])

                    if i == 3:
                        # Q for the core's own rows (tiles 0..3) -- early
                        for j in range(PAIRS):
                            psq = kp_ps.tile([128, 512], f32, name="ps_q",
                                             tag="ps_k")
                            for c2 in range(CCH // 2):
                                rhs = _swap12(xs8[:, 0:4, 2 * c2:2 * c2 + 2, :])
                                nc.tensor.matmul(
                                    psq, wq_sb[:, 2 * c2:2 * c2 + 2, ts(j, 128)],
                                    rhs, start=(c2 == 0),
                                    stop=(c2 == CCH // 2 - 1),
                                    perf_mode=PM.DoubleRow)
                            kq_evac(qT_all[:, j, :], psq,
                                    None if trivial_b else bq_sb[:, j:j + 1], j)
                    if i % 4 == 3:
                        g = i // 4
                        for j in range(PAIRS):
                            psn = kp_ps.tile([128, 512], f32, name="ps_k",
                                             tag="ps_k")
                            for c2 in range(CCH // 2):
                                rhs = _swap12(
                                    xs8[:, 4 * g:4 * g + 4, 2 * c2:2 * c2 + 2, :])
                                nc.tensor.matmul(
                                    psn, wk_sb[:, 2 * c2:2 * c2 + 2, ts(j, 128)],
                                    rhs, start=(c2 == 0),
                                    stop=(c2 == CCH // 2 - 1),
                                    perf_mode=PM.DoubleRow)
                            kq_evac(kT_all[:, j, ds(512 * g, 512)], psn,
                                    None if trivial_b else bk_sb[:, j:j + 1], j + 1)
        stB.close()

        # wp + p3 constants prefetch during attention (DMA is idle there)
        wp_sb = pR.tile([128, CCH, C], fp8)
        nc.sync.dma_start(out=wp_sb, in_=wp.ap())
        xq_sb = pR.tile([128, QT, C], f32)
        for i in range(QT):
            nc.sync.dma_start(out=xq_sb[:, i, :], in_=xq.ap()[ts(i, 128), :])
        ln2w_bc = pR.tile([128, C], f32)
        nc.sync.dma_start(out=ln2w_bc, in_=_bcast(ln2w.ap()))
        ln2b_bc = pR.tile([128, C], f32)
        nc.sync.dma_start(out=ln2b_bc, in_=_bcast(ln2b.ap()))
        bp_bc = pR.tile([128, C], f32)
        nc.sync.dma_start(out=bp_bc, in_=_bcast(bp.ap()))

        # ---- p2: per-pair attention; all probabilities fp8, all P@V DR ----
        scale = 1.0 / float(np.sqrt(DH))
        with nc.named_scope("p2_attn"), \
             tc.tile_pool(name="s_ps", bufs=3, space="PSUM") as s_ps, \
             tc.tile_pool(name="y_ps", bufs=1, space="PSUM") as y_ps, \
             tc.tile_pool(name="att_sb", bufs=3) as att_sb:
            for j in range(PAIRS):
                kT_j = kT_all[:, j, :]
                qT_j = qT_all[:, j, :]
                ps_y1 = y_ps.tile([DH + 1, 512], f32, name="ps_y1", tag="ps_y1")
                ps_y2 = y_ps.tile([DH + 1, 512], f32, name="ps_y2", tag="ps_y2")
                p8s = []

                def pv(e):
                    p8 = p8s[e]
                    for u in range(2):
                        nc.tensor.matmul(ps_y1 if u == 0 else ps_y2,
                                         v8[:, e, :, 2 * j + u, :],
                                         p8[:, :, u, :].bitcast(fp8),
                                         start=(e == 0), stop=(e == NT // 2 - 1),
                                         perf_mode=PM.DoubleRow)

                for e in range(NT // 2):
                    p8 = att_sb.tile([128, 2, 2, 512], i8, name="p8", tag="p8",
                                     bufs=3)
                    p8s.append(p8)
                    for par in range(2):
                        cidx = 2 * e + par
                        ps_s = s_ps.tile([128, 1024], f32, name="ps_s", tag="ps_s")
                        nc.tensor.matmul(ps_s[:, 0:512],
                                         kT_j[0:64, ts(cidx, 128)],
                                         qT_j[0:64, :], start=True, stop=True)
                        nc.tensor.matmul(ps_s[:, 512:1024],
                                         kT_j[64:128, ts(cidx, 128)],
                                         qT_j[64:128, :], start=True, stop=True,
                                         tile_position=(64, 0))
                        dst = p8[:, par, :, :]
                        if cidx in ACT_CIDX:
                            nc.scalar.activation(dst.bitcast(fp8), ps_s, AF.Exp,
                                                 scale=scale)
                        else:
                            nc.vector.tensor_scalar(out=dst, in0=ps_s,
                                                    scalar1=EA8 * scale,
                                                    scalar2=EB8,
                                                    op0=OP.mult, op1=OP.add)
                    if e >= 1:
                        pv(e - 1)
                pv(NT // 2 - 1)

                for u, ps_y in ((0, ps_y1), (1, ps_y2)):
                    # y out of PSUM (frees accumulator bank); 1/Z via a one-op
                    # magic-constant bf16 reciprocal; broadcast via PE outer
                    ycp = att_sb.tile([64, 512], f32, name="ycp", tag="ycp")
                    if u == 0:
                        nc.scalar.activation(ycp, ps_y[0:DH, :], AF.Identity)
                    else:
                        nc.vector.tensor_copy(ycp, ps_y[0:DH, :])
                    rsb = att_sb.tile([1, 512], i16, name="rsb", tag="rsb")
                    nc.vector.tensor_scalar(
                        out=rsb, in0=ps_y[DH:DH + 1, :].bitcast(i32),
                        scalar1=-(2.0 ** -16), scalar2=RCP_C,
                        op0=OP.mult, op1=OP.add)
                    # 1/Z broadcast reuses the drained y accumulator bank
                    bc = ps_y[0:DH, :]
                    nc.tensor.matmul(bc, ones64[:], rsb[:].bitcast(bf16),
                                     start=True, stop=True)
                    nc.vector.tensor_mul(ynT[64 * u:64 * u + 64, j, :],
                                         ycp, bc)
        stA.close()

        # ---- p3: attn projection + residual + LN2 + h2^T ----
        pD = st.enter_context(tc.tile_pool(name="pD", bufs=1, side="left"))
        x2 = pD.tile([128, QT, C], f32)
        h2Tb = pD.tile([128, CCH, QT, 128], f16)
        h2T8 = pD.tile([128, CCH // 2, QT, 128], fp8)
        bfc_sb = pD.tile([128, FT], f32)
        nc.sync.dma_start(out=bfc_sb, in_=bf_.ap())
        bm_bc = pD.tile([128, C], f32)
        nc.sync.dma_start(out=bm_bc, in_=_bcast(bm.ap()))
        wm8_all = pD.tile([128, FP8T // 2, 2, C], fp8)
        for a in range(FP8T // 2):
            nc.sync.dma_start(out=wm8_all[:, a, :, :], in_=wm8.ap()[a])

        def layer_norm2(x_t, out_ap):
            stats = stat.tile([128, 2, nc.vector.BN_STATS_DIM], f32,
                              name="stats2", tag="stats")
            nc.vector.bn_stats(out=stats[:, 0, :], in_=x_t[:, 0:512])
            nc.vector.bn_stats(out=stats[:, 1, :], in_=x_t[:, 512:1024])
            mv = stat.tile([128, nc.vector.BN_AGGR_DIM], f32, name="mv2",
                           tag="mv")
            nc.vector.bn_aggr(out=mv, in_=stats)
            rstd = stat.tile([128, 1], f32, name="rstd2", tag="rstd_f")
            nc.scalar.activation(rstd, mv[:, 1:2], AF.Sqrt, bias=eps_t)
            nc.vector.reciprocal(rstd, rstd)
            if trivial_ln2:
                nc.vector.tensor_scalar(out=out_ap, in0=x_t, scalar1=mv[:, 0:1],
                                        scalar2=rstd, op0=OP.subtract,
                                        op1=OP.mult)
            else:
                t1 = stat.tile([128, C], f32, name="t1", tag="ln_t1")
                nc.vector.tensor_scalar(out=t1, in0=x_t, scalar1=mv[:, 0:1],
                                        scalar2=rstd, op0=OP.subtract,
                                        op1=OP.mult)
                nc.vector.tensor_mul(t1, t1, ln2w_bc)
                nc.vector.tensor_add(out_ap, t1, ln2b_bc)

        with nc.named_scope("p3_proj_ln2"):
            with tc.tile_pool(name="ap_ps", bufs=2, space="PSUM") as ap_ps:
                h2_ts = []
                for i in range(QT):
                    xb_t = xq_sb[:, i, :]
                    nc.vector.tensor_add(xb_t, xb_t, bp_bc)
                    for n in range(C // 512):
                        ps = ap_ps.tile([128, 512], f32, name="ps_a", tag="ps_a")
                        for a in range(PAIRS // 2):
                            nc.tensor.matmul(ps, ynT[:, 2 * a:2 * a + 2, ts(i, 128)],
                                             wp_sb[:, 2 * a:2 * a + 2, ds(512 * n, 512)],
                                             start=(a == 0), stop=(a == PAIRS // 2 - 1),
                                             perf_mode=PM.DoubleRow)
                        nc.vector.tensor_add(x2[:, i, ds(512 * n, 512)], ps,
                                             xb_t[:, ds(512 * n, 512)])
                    h2_t = stream.tile([128, C], f16, name="h2_t", tag="h2_t", bufs=5)
                    layer_norm2(x2[:, i, :], h2_t)
                    h2_ts.append(h2_t)
                # batch the SBUF->SBUF transposes (they exclude other DMA
                # traffic; batched they share one exclusion window)
                for i in range(QT):
                    nc.scalar.dma_start_transpose(h2Tb[:, :, i, :], h2_ts[i][:])
                    nc.scalar.activation(h2T8[:, :, i, :], h2Tb[:, 0:CCH // 2, i, :],
                                         AF.Identity)

        # ---- p4: MLP fc + gelu ----
        # fold the mlp_proj bias into the residual copy while fc runs (DVE idle)
        for i in range(QT):
            nc.vector.tensor_add(x2[:, i, :], x2[:, i, :], bm_bc)
        stC.close()
        gTf8 = pD.tile([128, FP8T, TQ], fp8)
        gTff = pD.tile([128, FT - FP8T, TQ], f16)
        wmp = st.enter_context(tc.tile_pool(name="wm_all", bufs=1, side="right"))
        wmf_all = wmp.tile([128, FT - FP8T, C], f16)
        with nc.named_scope("p4_fc"):
            with tc.tile_pool(name="fc_ps", bufs=4, space="PSUM") as fc_ps, \
                 tc.tile_pool(name="wf_sb", bufs=4) as wf_pool:
                for t in range(FT):
                    wf8_t = wf_pool.tile([128, CCH // 2, 128], fp8, name="wf8_t",
                                         tag="wf8_t")
                    nc.sync.dma_start(out=wf8_t, in_=wf8.ap()[t])
                    wff_t = wf_pool.tile([128, CCH // 2, 128], f16, name="wff_t",
                                         tag="wff_t")
                    nc.sync.dma_start(out=wff_t, in_=wff.ap()[t])
                    if t < FT - FP8T:
                        nc.sync.dma_start(out=wmf_all[:, t, :],
                                          in_=wmf.ap()[ts(t, 128), :])
                    ps = fc_ps.tile([128, 512], f32, name="ps_f", tag="ps_f")
                    for c2 in range(2):
                        nc.tensor.matmul(ps, wf8_t[:, 2 * c2:2 * c2 + 2, :],
                                         h2T8[:, 2 * c2:2 * c2 + 2, 0:QT, :],
                                         start=(c2 == 0), stop=False,
                                         perf_mode=PM.DoubleRow)
                    for cx in range(CCH // 2):
                        nc.tensor.matmul(ps, wff_t[:, cx, :],
                                         h2Tb[:, CCH // 2 + cx, 0:QT, :],
                                         start=False, stop=(cx == CCH // 2 - 1))
                    if t < FP8T:
                        nc.scalar.activation(gTf8[:, t, :], ps, AF.Gelu_apprx_tanh,
                                             bias=bfc_sb[:, t:t + 1], scale=1.0)
                    else:
                        nc.scalar.activation(gTff[:, t - FP8T, :], ps,
                                             AF.Gelu_apprx_tanh,
                                             bias=bfc_sb[:, t:t + 1], scale=1.0)

        # ---- p5: MLP out projection (half fp8 DoubleRow, half f16) ----
        # i-outer: each token tile's accumulation finishes early so the
        # drain + output DMA of tile i overlaps tile i+1's matmuls
        with nc.named_scope("p5_mlp_out"):
            with tc.tile_pool(name="m_ps", bufs=4, space="PSUM") as m_ps, \
                 tc.tile_pool(name="out_sb", bufs=2) as out_pool:
                for i in range(QT):
                    ps_m = [m_ps.tile([128, 512], f32, name=f"ps_m{n}",
                                      tag="ps_m") for n in range(2)]
                    for a in range(FP8T // 2):
                        for n in range(C // 512):
                            nc.tensor.matmul(ps_m[n],
                                             gTf8[:, 2 * a:2 * a + 2, ts(i, 128)],
                                             wm8_all[:, a, :, ds(512 * n, 512)],
                                             start=(a == 0), stop=False,
                                             perf_mode=PM.DoubleRow)
                    for t in range(FT - FP8T):
                        last = t == FT - FP8T - 1
                        for n in range(C // 512):
                            nc.tensor.matmul(ps_m[n],
                                             gTff[:, t, ts(i, 128)],
                                             wmf_all[:, t, ds(512 * n, 512)],
                                             start=False, stop=last)
                    out_t = out_pool.tile([128, C], f32, name="out_t",
                                          tag="out_t")
                    for n in range(C // 512):
                        nc.vector.tensor_add(out_t[:, ds(512 * n, 512)],
                                             ps_m[n],
                                             x2[:, i, ds(512 * n, 512)])
                        nc.sync.dma_start(
                            out=out.ap()[ts(i, 128), ds(512 * n, 512)],
                            in_=out_t[:, ds(512 * n, 512)])


def _get_program(trivial_b, trivial_ln2):
    key = (trivial_b, trivial_ln2)
    if key not in _CACHED:
        _CACHED[key] = _build_program(trivial_b, trivial_ln2)
    return _CACHED[key]


def _fp8(a):
    return np.ascontiguousarray(np.asarray(a, np.float32)
                                .clip(-240, 240).astype(ml_dtypes.float8_e4m3))


def _tile_proj_weight(w):
    # [C, N] f32 -> [128, CCH, N] fp8 with partition = c % 128, chunk = c // 128
    w = np.asarray(w, np.float32).reshape(CCH, 128, -1)
    return _fp8(w.transpose(1, 0, 2))


def _prep_in_maps(inputs):
    fl = lambda a: np.ascontiguousarray(np.asarray(a, np.float32))
    x = fl(inputs["x"])
    ln1w = fl(inputs["ln1_w"])
    ln1b = fl(inputs["ln1_b"])
    attn_w = fl(inputs["attn_w"]) * ln1w[:, None]      # fold LN1 gamma
    battn = ln1b @ attn_w + fl(inputs["attn_b"])       # fold LN1 beta
    attn_w = attn_w - attn_w.mean(0, keepdims=True)    # fold mean removal
    wf_full = fl(inputs["fc_w"])  # [C, F]
    # wf tiled: [FT, 128(c), CCH, 128(f')]; chunks 0..3 fp8, 4..7 f16
    wf_t = np.ascontiguousarray(
        wf_full.reshape(CCH, 128, FT, 128).transpose(2, 1, 0, 3))
    wf8_t = _fp8(wf_t[:, :, 0:CCH // 2, :])
    wff_t = np.ascontiguousarray(wf_t[:, :, CCH // 2:, :].astype(np.float16))
    wm_full = fl(inputs["mlp_proj_w"])  # [F, C]
    wm8_t = _fp8(wm_full[0:FP8T * 128].reshape(FP8T // 2, 2, 128, C)
                 .transpose(0, 2, 1, 3))
    wmf_t = np.ascontiguousarray(wm_full[FP8T * 128:].astype(np.float16))
    pb = lambda b: np.ascontiguousarray(
        np.asarray(b, np.float32).reshape(-1, 128).T)  # [128, tiles]
    shared = {
        "ident": np.ascontiguousarray(np.eye(128, dtype=np.float32)
                                      .astype(ml_dtypes.bfloat16)),
        "wq": _tile_proj_weight(attn_w[:, 0:C]),
        "wk": _tile_proj_weight(attn_w[:, C:2 * C]),
        "wv": _tile_proj_weight(attn_w[:, 2 * C:3 * C]),
        "bq": pb(battn[0:C]), "bk": pb(battn[C:2 * C]),
        "bv": fl(battn[2 * C:3 * C]),
        "ln2w": fl(inputs["ln2_w"]), "ln2b": fl(inputs["ln2_b"]),
        "wp": _tile_proj_weight(inputs["attn_proj_w"]),
        "bp": fl(inputs["attn_proj_b"]),
        "wf8": wf8_t, "wff": wff_t, "bf": pb(inputs["fc_b"]),
        "wm8": wm8_t, "wmf": wmf_t,
        "bm": fl(inputs["mlp_proj_b"]),
    }
    in_maps = []
    for core in range(NCORES):
        b, r = core // GROUP, core % GROUP
        xb = np.roll(x[b], -TQ * r, axis=0)
        xb16 = xb.astype(ml_dtypes.bfloat16)
        in_maps.append({
            "xf": np.ascontiguousarray(xb16),
            "xft": np.ascontiguousarray(
                xb16.reshape(NT, 128, CCH, 128).transpose(0, 3, 2, 1)),
            "xq": np.ascontiguousarray(xb[0:TQ]),
            **shared,
        })
    return in_maps


def run(inputs, trace=False):
    fl = lambda a: np.asarray(a, np.float32)
    battn = fl(inputs["ln1_b"]) @ (fl(inputs["attn_w"])
                                   * fl(inputs["ln1_w"])[:, None]) \
        + fl(inputs["attn_b"])
    trivial_b = bool(np.all(np.abs(battn) < 1e-12))
    trivial_ln2 = bool(np.all(np.asarray(inputs["ln2_w"]) == 1.0)
                       and np.all(np.asarray(inputs["ln2_b"]) == 0.0))
    nc = _get_program(trivial_b, trivial_ln2)
    in_maps = _prep_in_maps(inputs)
    res = run_bass_kernel_spmd(nc, in_maps, core_ids=list(range(NCORES)),
                               trace=trace)
    out = np.empty((B, T, C), np.float32)
    for core in range(NCORES):
        b, r = core // GROUP, core % GROUP
        out[b, TQ * r:TQ * (r + 1)] = res.results[core]["out"]
    return out, res


def kernel(**inputs):
    out, _ = run(inputs, trace=False)
    return out


# revision 17
# speedup vs baseline: 1.1618x; 1.0109x over previous
"""Trainium2 Bass kernel for a GPT-2 style transformer block (pre-LN, no mask).

Reference shapes: x [B=2, T=2048, C=1024], H=16 heads, MLP hidden 4C=4096.

Sharding (8 NeuronCores): data-parallel over B (cores 0-3 -> batch 0,
cores 4-7 -> batch 1); within each 4-core group the 2048 query rows are
split 512 per core. Every core redundantly computes K and V for its full
batch from a replicated (rotated) copy of x, so no collectives are needed.

v3 changes vs v2:
  * p1: x is transposed straight from DRAM at t=0 (no LN->transpose
    serialization).  LN1's mean removal is folded into column-centered
    weights host-side; the per-token rstd is broadcast across partitions
    with a PE outer-product and applied in a single fused
    scalar_tensor_tensor that also converts to fp8.  Q is computed as
    soon as the core's own 4 token tiles are ready (not at the end).
  * p2: all softmax probabilities are fp8(e4m3): the scalar engine's Exp
    writes fp8 directly, the vector engine uses a one-op int8 Schraudolph
    exp (bitcast to e4m3), and every P@V matmul runs fp8 DoubleRow.
    1/Z uses a one-op magic-constant bf16 reciprocal.
  * p5: first half of mlp_proj runs fp8 DoubleRow (gelu writes fp8 for
    those tiles directly), second half stays f16.
"""

import numpy as np
import ml_dtypes

import concourse.bass as bass
import concourse.bacc as bacc
import concourse.tile as tile
from concourse import mybir
from concourse.bass import ts, ds
from concourse.bass_utils import run_bass_kernel_spmd

f32 = mybir.dt.float32
bf16 = mybir.dt.bfloat16
f16 = mybir.dt.float16
fp8 = mybir.dt.float8e4
i8 = mybir.dt.int8
i16 = mybir.dt.int16
i32 = mybir.dt.int32
AF = mybir.ActivationFunctionType
OP = mybir.AluOpType
PM = mybir.MatmulPerfMode

B, T, C, H = 2, 2048, 1024, 16
DH = C // H          # 64
F = 4 * C            # 4096
NCORES = 8
GROUP = 4            # cores per batch
TQ = T // GROUP      # 512 query rows per core
NT = T // 128        # 16 token tiles
CCH = C // 128       # 8 contraction chunks over C
PAIRS = H // 2       # 8 head pairs
FT = F // 128        # 32 hidden tiles
QT = TQ // 128       # 4 own-row tiles
FP8T = 8             # mlp_proj tiles done in fp8 DoubleRow (rest f16)

# int8 Schraudolph: e4m3(exp(x)) ~= bitcast_i8(round(EA8*x + EB8))
EA8 = 11.54156033    # 2^3 / ln 2
EB8 = 55.536         # 7*2^3 - 0.0579*2^3 (min-max relative error)
# bf16 magic reciprocal: 1/Z ~= bitcast_bf16(i16(RCP_C - bits_f32(Z)*2^-16))
RCP_C = 32499.0
# cidx values whose exp runs on the scalar engine (rest: DVE int8 fast-exp)
ACT_CIDX = frozenset((0, 2, 4, 6, 8, 10, 12, 14, 15))

_CACHED = {}


def _bcast(ap, parts=128):
    """DRAM AP for a 1-D tensor broadcast across `parts` partitions."""
    return bass.AP(tensor=ap.tensor, offset=ap.offset, ap=[[0, parts]] + list(ap.ap))


def _rep_mid(ap2d, n):
    """[128, W] SBUF AP -> [128, n, W] with stride-0 middle dim."""
    return bass.AP(tensor=ap2d.tensor, offset=ap2d.offset,
                   ap=[list(ap2d.ap[0]), [0, n]] + [list(a) for a in ap2d.ap[1:]])


def _swap12(ap):
    """Swap free dims 1 and 2 of a 4-dim AP (partition, a, b, c) -> (p, b, a, c)."""
    return bass.AP(tensor=ap.tensor, offset=ap.offset,
                   ap=[list(ap.ap[0]), list(ap.ap[2]), list(ap.ap[1]),
                       list(ap.ap[3])])


def _build_program(trivial_b, trivial_ln2):
    nc = bacc.Bacc("TRN2", target_bir_lowering=False, debug=False,
                   num_devices=NCORES)

    xf = nc.dram_tensor("xf", [T, C], bf16, kind="ExternalInput")
    xft = nc.dram_tensor("xft", [NT, 128, CCH, 128], bf16, kind="ExternalInput")
    xq = nc.dram_tensor("xq", [TQ, C], f32, kind="ExternalInput")
    ident = nc.dram_tensor("ident", [128, 128], bf16, kind="ExternalInput")
    # pre-tiled centered weights: [128 (c within chunk), CCH, out-features]
    wq = nc.dram_tensor("wq", [128, CCH, C], fp8, kind="ExternalInput")
    wk = nc.dram_tensor("wk", [128, CCH, C], fp8, kind="ExternalInput")
    wv = nc.dram_tensor("wv", [128, CCH, C], fp8, kind="ExternalInput")
    bqv = nc.dram_tensor("bq", [128, PAIRS], f32, kind="ExternalInput")
    bkv = nc.dram_tensor("bk", [128, PAIRS], f32, kind="ExternalInput")
    bvv = nc.dram_tensor("bv", [C], f32, kind="ExternalInput")
    ln2w = nc.dram_tensor("ln2w", [C], f32, kind="ExternalInput")
    ln2b = nc.dram_tensor("ln2b", [C], f32, kind="ExternalInput")
    wp = nc.dram_tensor("wp", [128, CCH, C], fp8, kind="ExternalInput")
    bp = nc.dram_tensor("bp", [C], f32, kind="ExternalInput")
    # wf pre-tiled per f'-tile, split: chunks 0..3 fp8 (DoubleRow), 4..7 f16
    wf8 = nc.dram_tensor("wf8", [FT, 128, CCH // 2, 128], fp8, kind="ExternalInput")
    wff = nc.dram_tensor("wff", [FT, 128, CCH // 2, 128], f16, kind="ExternalInput")
    bf_ = nc.dram_tensor("bf", [128, FT], f32, kind="ExternalInput")
    # mlp_proj: tiles 0..FP8T-1 as fp8 pairs, rest f16 rows
    wm8 = nc.dram_tensor("wm8", [FP8T // 2, 128, 2, C], fp8, kind="ExternalInput")
    wmf = nc.dram_tensor("wmf", [F - FP8T * 128, C], f16, kind="ExternalInput")
    bm = nc.dram_tensor("bm", [C], f32, kind="ExternalInput")
    out = nc.dram_tensor("out", [TQ, C], f32, kind="ExternalOutput")

    with tile.TileContext(nc) as tc:
        _emit(nc, tc, trivial_b, trivial_ln2,
              xf, xft, xq, ident, wq, wk, wv, bqv, bkv, bvv, ln2w, ln2b,
              wp, bp, wf8, wff, bf_, wm8, wmf, bm, out)
    nc.compile()
    return nc


def _emit(nc, tc, trivial_b, trivial_ln2,
          xf, xft, xq, ident, wq, wk, wv, bqv, bkv, bvv, ln2w, ln2b,
          wp, bp, wf8, wff, bf_, wm8, wmf, bm, out):
    from contextlib import ExitStack

    with ExitStack() as st:
        persist = st.enter_context(tc.tile_pool(name="persist", bufs=1))
        stat = st.enter_context(tc.tile_pool(name="stat", bufs=6))
        stream = st.enter_context(tc.tile_pool(name="stream", bufs=5))

        ones64 = persist.tile([1, 64], bf16)
        nc.vector.memset(ones64, 1.0)
        ones1 = persist.tile([1, 128], bf16)
        nc.vector.memset(ones1, 1.0)
        eps_t = persist.tile([128, 1], f32)
        nc.vector.memset(eps_t, 1e-5)
        rcp_t = persist.tile([1, 1], f32)
        nc.vector.memset(rcp_t, RCP_C)
        ident_sb = persist.tile([128, 128], bf16)
        nc.sync.dma_start(out=ident_sb, in_=ident.ap())

        # ---------------- pools (stack discipline per side) ----------------
        stA = st.enter_context(ExitStack())
        pA = stA.enter_context(tc.tile_pool(name="pA", bufs=1, side="left"))
        pW = st.enter_context(tc.tile_pool(name="pW", bufs=1, side="right"))
        stC = st.enter_context(ExitStack())
        pR = stC.enter_context(tc.tile_pool(name="pR", bufs=1, side="right"))
        stB = st.enter_context(ExitStack())
        pB = stB.enter_context(tc.tile_pool(name="pB", bufs=1, side="right"))

        # fp8 normalized+scaled activations (tile-major); xs8 and wk stay
        # alive through p2 (K groups 2,3 are computed there as PE gap-filler)
        xs8 = pA.tile([128, NT, CCH, 128], fp8)
        bc_all = pB.tile([128, NT, 128], bf16)

        wv_sb = pB.tile([128, CCH, C], fp8)
        wk_sb = pA.tile([128, CCH, C], fp8)
        wq_sb = pB.tile([128, CCH, C], fp8)
        if not trivial_b:
            bv_bc = pB.tile([128, C], f32)
            nc.sync.dma_start(out=bv_bc, in_=_bcast(bvv.ap()))
            bq_sb = pB.tile([128, PAIRS], f32)
            nc.sync.dma_start(out=bq_sb, in_=bqv.ap())
            bk_sb = pB.tile([128, PAIRS], f32)
            nc.sync.dma_start(out=bk_sb, in_=bkv.ap())

        kT_all = pA.tile([128, PAIRS, T], bf16)
        qT_all = pA.tile([128, PAIRS, TQ], bf16)
        v8 = pR.tile([128, NT // 2, 2, H, DH + 1], fp8)
        ynT = pR.tile([128, PAIRS, TQ], fp8)
        nc.vector.memset(v8[:, :, :, :, DH:DH + 1], 1.0)

        def kq_evac(dst, psn, bias_col, eng="act"):
            if eng == "act":
                if trivial_b:
                    nc.scalar.activation(dst, psn, AF.Identity)
                else:
                    nc.scalar.activation(dst, psn, AF.Identity, bias=bias_col)
            else:
                if trivial_b:
                    nc.vector.tensor_copy(dst, psn)
                else:
                    nc.vector.tensor_scalar(out=dst, in0=psn, scalar1=bias_col,
                                            scalar2=None, op0=OP.add)

        # ---- p1: stats + rstd-scale-to-fp8 + V/K/Q matmuls ----
        with nc.named_scope("p1_ln_v"):
            with tc.tile_pool(name="v_ps", bufs=4, space="PSUM") as v_ps, \
                 tc.tile_pool(name="kp_ps", bufs=2, space="PSUM") as kp_ps, \
                 tc.tile_pool(name="t_ps", bufs=2, space="PSUM") as t_ps:
                for i in range(NT):
                    x_t = stream.tile([128, C], bf16, name="x_t", tag="x_t", bufs=6)
                    nc.sync.dma_start(out=x_t, in_=xf.ap()[ts(i, 128), :])
                    xT_t = stream.tile([128, CCH, 128], bf16, name="xT_t",
                                       tag="xT_t", bufs=6)
                    nc.sync.dma_start(out=xT_t, in_=xft.ap()[i])
                    if i == 0:
                        nc.sync.dma_start(out=wv_sb, in_=wv.ap())
                    if i == 1:
                        nc.sync.dma_start(out=wk_sb, in_=wk.ap())
                    if i == 2:
                        nc.sync.dma_start(out=wq_sb, in_=wq.ap())
                    stats = stat.tile([128, 2, nc.vector.BN_STATS_DIM], f32,
                                      name="stats", tag="stats")
                    nc.vector.bn_stats(out=stats[:, 0, :], in_=x_t[:, 0:512])
                    nc.vector.bn_stats(out=stats[:, 1, :], in_=x_t[:, 512:1024])
                    mv = stat.tile([128, nc.vector.BN_AGGR_DIM], f32, name="mv",
                                   tag="mv")
                    nc.vector.bn_aggr(out=mv, in_=stats)
                    rstd_f = stat.tile([128, 1], f32, name="rstd_f", tag="rstd_f")
                    nc.scalar.activation(rstd_f, mv[:, 1:2], AF.Sqrt, bias=eps_t)
                    rstd = stat.tile([128, 1], bf16, name="rstd", tag="rstd")
                    with nc.allow_low_precision(reason="rstd bf16 for PE bcast"):
                        nc.vector.reciprocal(rstd, rstd_f)
                    # broadcast rstd across partitions: transpose + outer product
                    row_ps = t_ps.tile([1, 128], f32, name="row_ps", tag="row_ps", bufs=1)
                    nc.tensor.matmul(row_ps, rstd[:], ident_sb[:], start=True, stop=True)
                    row_sb = stat.tile([1, 128], bf16, name="row_sb", tag="row_sb")
                    nc.scalar.activation(row_sb, row_ps, AF.Identity)
                    bc_ps = t_ps.tile([128, 128], f32, name="bc_ps", tag="bc_ps", bufs=1)
                    nc.tensor.matmul(bc_ps, ones1[:], row_sb[:], start=True, stop=True)
                    nc.scalar.activation(bc_all[:, i, :], bc_ps, AF.Identity)
                    # fused normalize-scale-quantize: xs8 = (xT * rstd_bcast) fp8
                    nc.vector.tensor_mul(xs8[:, i, :, :], xT_t,
                                         _rep_mid(bc_all[:, i, :], CCH))

                    # V for this tile
                    pss = [v_ps.tile([128, 512], f32, name=f"ps_v{n}", tag="ps_v")
                           for n in range(2)]
                    for c2 in range(CCH // 2):
                        for n in range(C // 512):
                            nc.tensor.matmul(pss[n], xs8[:, i, 2 * c2:2 * c2 + 2, :],
                                             wv_sb[:, 2 * c2:2 * c2 + 2, ds(512 * n, 512)],
                                             start=(c2 == 0), stop=(c2 == CCH // 2 - 1),
                                             perf_mode=PM.DoubleRow)
                    e_, par = i // 2, i % 2
                    for n in range(C // 512):
                        dst = v8[:, e_, par, 8 * n:8 * n + 8, 0:DH]
                        if trivial_b:
                            nc.scalar.activation(dst, pss[n], AF.Identity)
                        else:
                            nc.vector.tensor_add(dst, pss[n],
                                                 bv_bc[:, ds(512 * n, 512)])

                    if i == 3:
                        # Q for the core's own rows (tiles 0..3) -- early
                        for j in range(PAIRS):
                            psq = kp_ps.tile([128, 512], f32, name="ps_q",
                                             tag="ps_k")
                            for c2 in range(CCH // 2):
                                rhs = _swap12(xs8[:, 0:4, 2 * c2:2 * c2 + 2, :])
                                nc.tensor.matmul(
                                    psq, wq_sb[:, 2 * c2:2 * c2 + 2, ts(j, 128)],
                                    rhs, start=(c2 == 0),
                                    stop=(c2 == CCH // 2 - 1),
                                    perf_mode=PM.DoubleRow)
                            kq_evac(qT_all[:, j, :], psq,
                                    None if trivial_b else bq_sb[:, j:j + 1])
                    if i % 4 == 3:
                        g = i // 4
                        for j in range(PAIRS):
                            psn = kp_ps.tile([128, 512], f32, name="ps_k",
                                             tag="ps_k")
                            for c2 in range(CCH // 2):
                                rhs = _swap12(
                                    xs8[:, 4 * g:4 * g + 4, 2 * c2:2 * c2 + 2, :])
                                nc.tensor.matmul(
                                    psn, wk_sb[:, 2 * c2:2 * c2 + 2, ts(j, 128)],
                                    rhs, start=(c2 == 0),
                                    stop=(c2 == CCH // 2 - 1),
                                    perf_mode=PM.DoubleRow)
                            kq_evac(kT_all[:, j, ds(512 * g, 512)], psn,
                                    None if trivial_b else bk_sb[:, j:j + 1])
        stB.close()

        # wp + p3 constants + fc fp8 weights prefetch during attention
        # (DMA is idle there; relieves p4's weight-stream contention)
        wp_sb = pR.tile([128, CCH, C], fp8)
        nc.sync.dma_start(out=wp_sb, in_=wp.ap())
        wf8_all = pW.tile([128, FT, CCH // 2, 128], fp8)
        for t in range(FT):
            nc.sync.dma_start(out=wf8_all[:, t, :, :], in_=wf8.ap()[t])
        WFP = 12
        wff_pre = pW.tile([128, WFP, CCH // 2, 128], f16)
        for t in range(WFP):
            nc.sync.dma_start(out=wff_pre[:, t, :, :], in_=wff.ap()[t])
        xq_sb = pR.tile([128, QT, C], f32)
        for i in range(QT):
            nc.sync.dma_start(out=xq_sb[:, i, :], in_=xq.ap()[ts(i, 128), :])
        ln2w_bc = pR.tile([128, C], f32)
        nc.sync.dma_start(out=ln2w_bc, in_=_bcast(ln2w.ap()))
        ln2b_bc = pR.tile([128, C], f32)
        nc.sync.dma_start(out=ln2b_bc, in_=_bcast(ln2b.ap()))
        bp_bc = pR.tile([128, C], f32)
        nc.sync.dma_start(out=bp_bc, in_=_bcast(bp.ap()))

        # ---- p2: per-pair attention; all probabilities fp8, all P@V DR ----
        scale = 1.0 / float(np.sqrt(DH))
        with nc.named_scope("p2_attn"), \
             tc.tile_pool(name="s_ps", bufs=3, space="PSUM") as s_ps, \
             tc.tile_pool(name="y_ps", bufs=1, space="PSUM") as y_ps, \
             tc.tile_pool(name="att_sb", bufs=3) as att_sb:
            for j in range(PAIRS):
                kT_j = kT_all[:, j, :]
                qT_j = qT_all[:, j, :]
                ps_y1 = y_ps.tile([DH + 1, 512], f32, name="ps_y1", tag="ps_y1")
                ps_y2 = y_ps.tile([DH + 1, 512], f32, name="ps_y2", tag="ps_y2")
                p8s = []

                def pv(e):
                    p8 = p8s[e]
                    for u in range(2):
                        nc.tensor.matmul(ps_y1 if u == 0 else ps_y2,
                                         v8[:, e, :, 2 * j + u, :],
                                         p8[:, :, u, :].bitcast(fp8),
                                         start=(e == 0), stop=(e == NT // 2 - 1),
                                         perf_mode=PM.DoubleRow)

                for e in range(NT // 2):
                    p8 = att_sb.tile([128, 2, 2, 512], i8, name="p8", tag="p8",
                                     bufs=5)
                    p8s.append(p8)
                    for par in range(2):
                        cidx = 2 * e + par
                        ps_s = s_ps.tile([128, 1024], f32, name="ps_s", tag="ps_s")
                        nc.tensor.matmul(ps_s[:, 0:512],
                                         kT_j[0:64, ts(cidx, 128)],
                                         qT_j[0:64, :], start=True, stop=True)
                        nc.tensor.matmul(ps_s[:, 512:1024],
                                         kT_j[64:128, ts(cidx, 128)],
                                         qT_j[64:128, :], start=True, stop=True,
                                         tile_position=(64, 0))
                        dst = p8[:, par, :, :]
                        if cidx in ACT_CIDX:
                            nc.scalar.activation(dst.bitcast(fp8), ps_s, AF.Exp,
                                                 scale=scale)
                        else:
                            nc.vector.tensor_scalar(out=dst, in0=ps_s,
                                                    scalar1=EA8 * scale,
                                                    scalar2=EB8,
                                                    op0=OP.mult, op1=OP.add)
                    if e >= 1:
                        pv(e - 1)
                pv(NT // 2 - 1)

                for u, ps_y in ((0, ps_y1), (1, ps_y2)):
                    # y out of PSUM (frees accumulator bank); 1/Z via a one-op
                    # magic-constant bf16 reciprocal; broadcast via PE outer
                    ycp = att_sb.tile([64, 512], f32, name="ycp", tag="ycp")
                    if u == 0:
                        nc.scalar.activation(ycp, ps_y[0:DH, :], AF.Identity)
                    else:
                        nc.vector.tensor_copy(ycp, ps_y[0:DH, :])
                    rsb = att_sb.tile([1, 512], i16, name="rsb", tag="rsb")
                    if u == 0:
                        nc.scalar.activation(rsb, ps_y[DH:DH + 1, :].bitcast(i32),
                                             AF.Identity, bias=rcp_t,
                                             scale=-(2.0 ** -16))
                    else:
                        nc.vector.tensor_scalar(
                            out=rsb, in0=ps_y[DH:DH + 1, :].bitcast(i32),
                            scalar1=-(2.0 ** -16), scalar2=RCP_C,
                            op0=OP.mult, op1=OP.add)
                    # 1/Z broadcast reuses the drained y accumulator bank
                    bc = ps_y[0:DH, :]
                    nc.tensor.matmul(bc, ones64[:], rsb[:].bitcast(bf16),
                                     start=True, stop=True)
                    nc.vector.tensor_mul(ynT[64 * u:64 * u + 64, j, :],
                                         ycp, bc)
        stA.close()

        # ---- p3: attn projection + residual + LN2 + h2^T ----
        pD = st.enter_context(tc.tile_pool(name="pD", bufs=1, side="left"))
        x2 = pD.tile([128, QT, C], f32)
        h2Tb = pD.tile([128, CCH, QT, 128], f16)
        h2T8 = pD.tile([128, CCH // 2, QT, 128], fp8)
        bfc_sb = pD.tile([128, FT], f32)
        nc.sync.dma_start(out=bfc_sb, in_=bf_.ap())
        bm_bc = pD.tile([128, C], f32)
        nc.sync.dma_start(out=bm_bc, in_=_bcast(bm.ap()))
        wm8_all = pD.tile([128, FP8T // 2, 2, C], fp8)
        for a in range(FP8T // 2):
            nc.sync.dma_start(out=wm8_all[:, a, :, :], in_=wm8.ap()[a])

        def layer_norm2(x_t, out_ap):
            stats = stat.tile([128, 2, nc.vector.BN_STATS_DIM], f32,
                              name="stats2", tag="stats")
            nc.vector.bn_stats(out=stats[:, 0, :], in_=x_t[:, 0:512])
            nc.vector.bn_stats(out=stats[:, 1, :], in_=x_t[:, 512:1024])
            mv = stat.tile([128, nc.vector.BN_AGGR_DIM], f32, name="mv2",
                           tag="mv")
            nc.vector.bn_aggr(out=mv, in_=stats)
            rstd = stat.tile([128, 1], f32, name="rstd2", tag="rstd_f")
            nc.scalar.activation(rstd, mv[:, 1:2], AF.Sqrt, bias=eps_t)
            nc.vector.reciprocal(rstd, rstd)
            if trivial_ln2:
                nc.vector.tensor_scalar(out=out_ap, in0=x_t, scalar1=mv[:, 0:1],
                                        scalar2=rstd, op0=OP.subtract,
                                        op1=OP.mult)
            else:
                t1 = stat.tile([128, C], f32, name="t1", tag="ln_t1")
                nc.vector.tensor_scalar(out=t1, in0=x_t, scalar1=mv[:, 0:1],
                                        scalar2=rstd, op0=OP.subtract,
                                        op1=OP.mult)
                nc.vector.tensor_mul(t1, t1, ln2w_bc)
                nc.vector.tensor_add(out_ap, t1, ln2b_bc)

        fc_stack = ExitStack()
        fc_ps = fc_stack.enter_context(
            tc.tile_pool(name="fc_ps", bufs=4, space="PSUM"))
        wf_pool = fc_stack.enter_context(tc.tile_pool(name="wf_sb", bufs=4))

        with nc.named_scope("p3_proj_ln2"):
            with tc.tile_pool(name="ap_ps", bufs=3, space="PSUM") as ap_ps:
                h2_ts = []
                for i in range(QT):
                    xb_t = xq_sb[:, i, :]
                    nc.vector.tensor_add(xb_t, xb_t, bp_bc)
                    for n in range(C // 512):
                        ps = ap_ps.tile([128, 512], f32, name="ps_a", tag="ps_a")
                        for a in range(PAIRS // 2):
                            nc.tensor.matmul(ps, ynT[:, 2 * a:2 * a + 2, ts(i, 128)],
                                             wp_sb[:, 2 * a:2 * a + 2, ds(512 * n, 512)],
                                             start=(a == 0), stop=(a == PAIRS // 2 - 1),
                                             perf_mode=PM.DoubleRow)
                        nc.vector.tensor_add(x2[:, i, ds(512 * n, 512)], ps,
                                             xb_t[:, ds(512 * n, 512)])
                    h2_t = stream.tile([128, C], f16, name="h2_t", tag="h2_t", bufs=5)
                    layer_norm2(x2[:, i, :], h2_t)
                    h2_ts.append(h2_t)
                # batch the SBUF->SBUF transposes (they exclude other DMA
                # traffic; batched they share one exclusion window)
                for i in range(QT):
                    nc.scalar.dma_start_transpose(h2Tb[:, :, i, :], h2_ts[i][:])
                    nc.scalar.activation(h2T8[:, :, i, :], h2Tb[:, 0:CCH // 2, i, :],
                                         AF.Identity)

        # ---- p4: MLP fc + gelu ----
        # fold the mlp_proj bias into the residual copy while fc runs (DVE idle)
        for i in range(QT):
            nc.vector.tensor_add(x2[:, i, :], x2[:, i, :], bm_bc)
        stC.close()
        gTf8 = pD.tile([128, FP8T, TQ], fp8)
        gTff = pD.tile([128, FT - FP8T, TQ], f16)
        wmp = st.enter_context(tc.tile_pool(name="wm_all", bufs=1, side="right"))
        wmf_all = wmp.tile([128, FT - FP8T, C], f16)
        with nc.named_scope("p4_fc"):
            with fc_stack:
                for t in range(FT):
                    wf8_t = wf8_all[:, t, :, :]
                    if t < WFP:
                        wff_t = wff_pre[:, t, :, :]
                    else:
                        wff_t = wf_pool.tile([128, CCH // 2, 128], f16,
                                             name="wff_t", tag="wff_t")
                        nc.sync.dma_start(out=wff_t, in_=wff.ap()[t])
                    if t < FT - FP8T:
                        nc.sync.dma_start(out=wmf_all[:, t, :],
                                          in_=wmf.ap()[ts(t, 128), :])
                    ps = fc_ps.tile([128, 512], f32, name="ps_f", tag="ps_f")
                    # early tiles run in token-halves so fc can start while
                    # the later h2 tiles are still being normalized
                    halves = ((0, 2), (2, 2)) if t < 8 else ((0, 4),)
                    for hx, (lo, ni) in enumerate(halves):
                        first = hx == 0
                        lasth = hx == len(halves) - 1
                        for c2 in range(2):
                            nc.tensor.matmul(
                                ps[:, ds(128 * lo, 128 * ni)],
                                wf8_t[:, 2 * c2:2 * c2 + 2, :],
                                h2T8[:, 2 * c2:2 * c2 + 2, lo:lo + ni, :],
                                start=(first and c2 == 0), stop=False,
                                perf_mode=PM.DoubleRow)
                        for cx in range(CCH // 2):
                            nc.tensor.matmul(
                                ps[:, ds(128 * lo, 128 * ni)],
                                wff_t[:, cx, :],
                                h2Tb[:, CCH // 2 + cx, lo:lo + ni, :],
                                start=False,
                                stop=(lasth and cx == CCH // 2 - 1))
                    if t < FP8T:
                        nc.scalar.activation(gTf8[:, t, :], ps, AF.Gelu_apprx_tanh,
                                             bias=bfc_sb[:, t:t + 1], scale=1.0)
                    else:
                        nc.scalar.activation(gTff[:, t - FP8T, :], ps,
                                             AF.Gelu_apprx_tanh,
                                             bias=bfc_sb[:, t:t + 1], scale=1.0)

        # ---- p5: MLP out projection (half fp8 DoubleRow, half f16) ----
        # i-outer: each token tile's accumulation finishes early so the
        # drain + output DMA of tile i overlaps tile i+1's matmuls
        with nc.named_scope("p5_mlp_out"):
            with tc.tile_pool(name="m_ps", bufs=4, space="PSUM") as m_ps, \
                 tc.tile_pool(name="out_sb", bufs=2) as out_pool:
                for i in range(QT):
                    ps_m = [m_ps.tile([128, 512], f32, name=f"ps_m{n}",
                                      tag="ps_m") for n in range(2)]
                    for a in range(FP8T // 2):
                        for n in range(C // 512):
                            nc.tensor.matmul(ps_m[n],
                                             gTf8[:, 2 * a:2 * a + 2, ts(i, 128)],
                                             wm8_all[:, a, :, ds(512 * n, 512)],
                                             start=(a == 0), stop=False,
                                             perf_mode=PM.DoubleRow)
                    for t in range(FT - FP8T):
                        last = t == FT - FP8T - 1
                        for n in range(C // 512):
                            nc.tensor.matmul(ps_m[n],
                                             gTff[:, t, ts(i, 128)],
                                             wmf_all[:, t, ds(512 * n, 512)],
                                             start=False, stop=last)
                    out_t = out_pool.tile([128, C], f32, name="out_t",
                                          tag="out_t")
                    for n in range(C // 512):
                        nc.vector.tensor_add(out_t[:, ds(512 * n, 512)],
                                             ps_m[n],
                                             x2[:, i, ds(512 * n, 512)])
                        nc.sync.dma_start(
                            out=out.ap()[ts(i, 128), ds(512 * n, 512)],
                            in_=out_t[:, ds(512 * n, 512)])


def _get_program(trivial_b, trivial_ln2):
    key = (trivial_b, trivial_ln2)
    if key not in _CACHED:
        _CACHED[key] = _build_program(trivial_b, trivial_ln2)
    return _CACHED[key]


def _fp8(a):
    return np.ascontiguousarray(np.asarray(a, np.float32)
                                .clip(-240, 240).astype(ml_dtypes.float8_e4m3))


def _tile_proj_weight(w):
    # [C, N] f32 -> [128, CCH, N] fp8 with partition = c % 128, chunk = c // 128
    w = np.asarray(w, np.float32).reshape(CCH, 128, -1)
    return _fp8(w.transpose(1, 0, 2))


def _prep_in_maps(inputs):
    fl = lambda a: np.ascontiguousarray(np.asarray(a, np.float32))
    x = fl(inputs["x"])
    ln1w = fl(inputs["ln1_w"])
    ln1b = fl(inputs["ln1_b"])
    attn_w = fl(inputs["attn_w"]) * ln1w[:, None]      # fold LN1 gamma
    battn = ln1b @ attn_w + fl(inputs["attn_b"])       # fold LN1 beta
    attn_w = attn_w - attn_w.mean(0, keepdims=True)    # fold mean removal
    wf_full = fl(inputs["fc_w"])  # [C, F]
    # wf tiled: [FT, 128(c), CCH, 128(f')]; chunks 0..3 fp8, 4..7 f16
    wf_t = np.ascontiguousarray(
        wf_full.reshape(CCH, 128, FT, 128).transpose(2, 1, 0, 3))
    wf8_t = _fp8(wf_t[:, :, 0:CCH // 2, :])
    wff_t = np.ascontiguousarray(wf_t[:, :, CCH // 2:, :].astype(np.float16))
    wm_full = fl(inputs["mlp_proj_w"])  # [F, C]
    wm8_t = _fp8(wm_full[0:FP8T * 128].reshape(FP8T // 2, 2, 128, C)
                 .transpose(0, 2, 1, 3))
    wmf_t = np.ascontiguousarray(wm_full[FP8T * 128:].astype(np.float16))
    pb = lambda b: np.ascontiguousarray(
        np.asarray(b, np.float32).reshape(-1, 128).T)  # [128, tiles]
    shared = {
        "ident": np.ascontiguousarray(np.eye(128, dtype=np.float32)
                                      .astype(ml_dtypes.bfloat16)),
        "wq": _tile_proj_weight(attn_w[:, 0:C]),
        "wk": _tile_proj_weight(attn_w[:, C:2 * C]),
        "wv": _tile_proj_weight(attn_w[:, 2 * C:3 * C]),
        "bq": pb(battn[0:C]), "bk": pb(battn[C:2 * C]),
        "bv": fl(battn[2 * C:3 * C]),
        "ln2w": fl(inputs["ln2_w"]), "ln2b": fl(inputs["ln2_b"]),
        "wp": _tile_proj_weight(inputs["attn_proj_w"]),
        "bp": fl(inputs["attn_proj_b"]),
        "wf8": wf8_t, "wff": wff_t, "bf": pb(inputs["fc_b"]),
        "wm8": wm8_t, "wmf": wmf_t,
        "bm": fl(inputs["mlp_proj_b"]),
    }
    in_maps = []
    for core in range(NCORES):
        b, r = core // GROUP, core % GROUP
        xb = np.roll(x[b], -TQ * r, axis=0)
        xb16 = xb.astype(ml_dtypes.bfloat16)
        in_maps.append({
            "xf": np.ascontiguousarray(xb16),
            "xft": np.ascontiguousarray(
                xb16.reshape(NT, 128, CCH, 128).transpose(0, 3, 2, 1)),
            "xq": np.ascontiguousarray(xb[0:TQ]),
            **shared,
        })
    return in_maps


def run(inputs, trace=False):
    fl = lambda a: np.asarray(a, np.float32)
    battn = fl(inputs["ln1_b"]) @ (fl(inputs["attn_w"])
                                   * fl(inputs["ln1_w"])[:, None]) \
        + fl(inputs["attn_b"])
    trivial_b = bool(np.all(np.abs(battn) < 1e-12))
    trivial_ln2 = bool(np.all(np.asarray(inputs["ln2_w"]) == 1.0)
                       and np.all(np.asarray(inputs["ln2_b"]) == 0.0))
    nc = _get_program(trivial_b, trivial_ln2)
    in_maps = _prep_in_maps(inputs)
    res = run_bass_kernel_spmd(nc, in_maps, core_ids=list(range(NCORES)),
                               trace=trace)
    out = np.empty((B, T, C), np.float32)
    for core in range(NCORES):
        b, r = core // GROUP, core % GROUP
        out[b, TQ * r:TQ * (r + 1)] = res.results[core]["out"]
    return out, res


def kernel(**inputs):
    out, _ = run(inputs, trace=False)
    return out
